# revision 13
# baseline (speedup 1.0000x reference)
"""DeltaNet block as a Bass/Tile SPMD kernel on 8 TRN2 NeuronCores.

Sharding: one (batch, head) pair per core (B=2 x NH=4 = 8 cores).

Host->device traffic is minimized: each core uploads only a 1/4-channel
slice of its batch's x^T, half of its head's packed weights (the batch-pair
core uploads the other half), and a quarter of its group's gate-MLP w1 slice.
On-device AllGathers (4-way for x, pair-wise for weights) reconstruct the
full operands over NeuronLink. Depthwise-conv/FIR diagonal matrices are
built on device from compact tap vectors (ident * tap column).

Per core: q/k/v/beta projections (head slice) + causal convs + silu, l2norm,
chunkwise delta rule (C=128 chunks, product-form unit-triangular inverse),
FIR convs (PE diagonal-matmul), branch stats, a 4-core AllReduce of the
stats rows, the gate MLP (4 of 16 hidden tiles x all tokens, with a
partial-logit ReduceScatter), AllGather of gate weights, channel-major
4-way mix + RMSNorm, the output projection and a ReduceScatter that both
sums heads and splits tokens. Output is returned bf16 and upcast on host.

All matmul operands are bf16 (fp32 PSUM accumulation).
"""

import sys
import threading

import numpy as np

if "/opt/trn_rl_repo" not in sys.path:
    sys.path.insert(0, "/opt/trn_rl_repo")

import ml_dtypes

BF16NP = ml_dtypes.bfloat16
F32NP = np.float32

B, L, H = 2, 2048, 1024
NH, DK, DV = 4, 256, 256
CONV_K, FIR_S, FIR_L = 4, 3, 63
MLP_H = 2 * H
C = 128
NCH = L // C
PAD = 64
TOK = L
TSL = L // 4
NC4 = TOK // 512

# packed-weight column offsets inside cpack
CP_IDENT = 0
CP_MSL = 128
CP_MSU = 256
CP_MSUD = 384
CP_ONES4 = 512
CP_SUMSEL = 528          # 8 blocks x 48
CP_W2 = 912              # 4 blocks x 16
CP_BSUM = 976            # [16,16]
CP_HSEL = 992            # [16,128]
CP_CTAPS = 1120          # 6 tensors x 4 taps
CP_FSTAPS = 1144         # 2 kt x 3 taps
CP_FLTAPS = 1150         # 2 kt x 63 taps
W_CPACK = 1280
W_WPROJ = 7168
W_WOS = 2048
W_WH = W_WPROJ + W_WOS + W_CPACK  # 10496

# single fused per-core input blob (bf16 elements)
O_XQ = 0
O_WH = O_XQ + 256 * TOK          # 524288
O_W1 = O_WH + 64 * W_WH          # 1196032
O_FP = O_W1 + 256 * 1152         # 1490944
NBLOB = O_FP + 128 * 7           # 1491840

_CACHE = {}


def _build_program():
    from contextlib import ExitStack

    import concourse.bacc as bacc
    import concourse.mybir as mybir
    import concourse.tile as tile

    dt = mybir.dt
    BF = dt.bfloat16
    FP = dt.float32
    AF = mybir.ActivationFunctionType
    OP = mybir.AluOpType

    nc = bacc.Bacc("TRN2", target_bir_lowering=False, debug=False, num_devices=8)

    for v in (1e-6, 1e-5):
        t = nc.alloc_sbuf_tensor(f"const-float32-{v}", [128, 1], FP)
        nc.gpsimd.memset(t.ap(), v)
        nc.const_aps.aps[(FP, v)] = t.ap()
    nc.all_engine_barrier()

    d_blob = nc.dram_tensor("blob", [1, NBLOB], BF, kind="ExternalInput")
    d_out = nc.dram_tensor("out", [TSL, H], BF, kind="ExternalOutput")

    rg4 = [[0, 1, 2, 3], [4, 5, 6, 7]]
    rgp = [[0, 4], [1, 5], [2, 6], [3, 7]]

    with tile.TileContext(nc) as tc, ExitStack() as es:
        cst = es.enter_context(tc.tile_pool(name="cst", bufs=1))
        per = es.enter_context(tc.tile_pool(name="per", bufs=1))
        scr = es.enter_context(tc.tile_pool(name="scr", bufs=3))
        ps_a = es.enter_context(tc.tile_pool(name="ps_a", bufs=2, space="PSUM"))
        ps_b = es.enter_context(tc.tile_pool(name="ps_b", bufs=4, space="PSUM"))
        ps_t = es.enter_context(tc.tile_pool(name="ps_t", bufs=2, space="PSUM"))
        dram = es.enter_context(tc.tile_pool(name="dram", bufs=1, space="DRAM"))

        # ============ Stage 0: assemble inputs via on-device AllGather =======
        i_xq = dram.tile([256, TOK], BF)
        i_wh = dram.tile([64, W_WH], BF)
        i_w1 = dram.tile([256, 1152], BF)
        g_xT = dram.tile([1024, TOK], BF)
        g_w = dram.tile([128, W_WH], BF)
        g_w1 = dram.tile([512, 1152], BF)
        nc.sync.dma_start(i_xq[:], d_blob.ap()[0:1, O_XQ:O_WH])
        nc.sync.dma_start(i_wh[:], d_blob.ap()[0:1, O_WH:O_W1])
        nc.sync.dma_start(i_w1[:], d_blob.ap()[0:1, O_W1:O_FP])
        nc.gpsimd.collective_compute(
            "AllGather", mybir.AluOpType.bypass, replica_groups=rg4,
            ins=[i_xq[:]], outs=[g_xT[:]])
        nc.gpsimd.collective_compute(
            "AllGather", mybir.AluOpType.bypass, replica_groups=rgp,
            ins=[i_wh[:]], outs=[g_w[:]])
        nc.gpsimd.collective_compute(
            "AllGather", mybir.AluOpType.bypass, replica_groups=rgp,
            ins=[i_w1[:]], outs=[g_w1[:]])

        cpack = cst.tile([128, W_CPACK], BF)
        nc.sync.dma_start(cpack[:], g_w[:, W_WPROJ + W_WOS: W_WH])
        fpbf = cst.tile([128, 7], BF)
        nc.sync.dma_start(fpbf[:], d_blob.ap()[0:1, O_FP:NBLOB])
        fpack = cst.tile([128, 7], FP)
        nc.vector.tensor_copy(fpack[:], fpbf[:])
        wos = cst.tile([128, W_WOS], BF)
        nc.sync.dma_start(wos[:], g_w[:, W_WPROJ: W_WPROJ + W_WOS])

        ident = cpack[:, CP_IDENT:CP_IDENT + 128]
        msl = cpack[:, CP_MSL:CP_MSL + 128]
        msu = cpack[:, CP_MSU:CP_MSU + 128]
        msud = cpack[:, CP_MSUD:CP_MSUD + 128]
        ones4 = cpack[:, CP_ONES4:CP_ONES4 + 16]
        sumsel = cpack[:, CP_SUMSEL:CP_SUMSEL + 384]
        w2my = cpack[:, CP_W2:CP_W2 + 64]
        bsum = cpack[0:16, CP_BSUM:CP_BSUM + 16]
        hsel = cpack[0:16, CP_HSEL:CP_HSEL + 128]
        b1c = fpack[:, 0:4]
        b2c = fpack[0:16, 4:5]
        onwc = fpack[:, 5:7]

        # fp32 copy of the tap columns (scalar operands must be fp32)
        tapf = cst.tile([128, W_CPACK - CP_CTAPS], FP)
        nc.vector.tensor_copy(tapf[:], cpack[:, CP_CTAPS:W_CPACK])

        def build_diag(dst, col):
            # dst[128,128] = diag(tap column `col` of cpack); alternate engines
            scol = tapf[:, col - CP_CTAPS: col - CP_CTAPS + 1]
            if col % 2 == 0:
                nc.scalar.activation(dst, ident, AF.Copy, scale=scol)
            else:
                nc.vector.tensor_scalar_mul(dst, ident, scol)

        v_cm = per.tile([128, 2 * (PAD + TOK)], BF)
        fir_s = per.tile([128, 2 * TOK], BF)
        fir_l = per.tile([128, 2 * TOK], BF)
        delta_cm = per.tile([128, 2 * TOK], BF)
        brow = per.tile([1, TOK], BF)
        beta_tm = per.tile([128, NCH], FP)
        S_bf = per.tile([128, 2 * DV], BF)
        stats32 = per.tile([128, TOK], BF)
        stats_sb = per.tile([128, TOK], BF)
        o_mix = per.tile([128, 2 * TOK], BF)

        nc.gpsimd.memset(stats32[:], 0.0)
        nc.gpsimd.memset(stats_sb[:], 0.0)
        nc.vector.memset(S_bf[:], 0.0)
        nc.gpsimd.memset(v_cm[:, 0:PAD], 0.0)
        nc.gpsimd.memset(v_cm[:, PAD + TOK:PAD + TOK + PAD], 0.0)

        # ================= Stage 1+2: projections, convs, silu ================
        mid = tc.tile_pool(name="mid", bufs=1)
        midp = mid.__enter__()
        q_cm = midp.tile([128, 2 * TOK], BF)
        delta_tm = midp.tile([128, NCH * DV], BF)
        k_cm = midp.tile([128, 2 * TOK], BF)
        kb_cm = midp.tile([128, 2 * TOK], BF)
        bb = midp.tile([128, TOK], BF)

        with tc.tile_pool(name="stg1", bufs=1) as stg1:
            wproj = stg1.tile([128, 8 * 7 * 128], BF)
            nc.sync.dma_start(wproj[:], g_w[:, 0:W_WPROJ])
            cdiag = stg1.tile([128, 6 * CONV_K * 128], BF)
            for s in range(6 * CONV_K):
                build_diag(cdiag[:, s * 128:(s + 1) * 128], CP_CTAPS + s)
            qkvb = stg1.tile([128, 6 * (PAD + TOK)], BF)
            for mt in range(6):
                nc.gpsimd.memset(
                    qkvb[:, mt * (PAD + TOK): mt * (PAD + TOK) + PAD], 0.0)

            for n in range(NC4):
                xb = stg1.tile([128, 8 * 512], BF, tag="xb", bufs=2)
                for kt in range(8):
                    nc.sync.dma_start(
                        xb[:, kt * 512:(kt + 1) * 512],
                        g_xT[kt * 128:(kt + 1) * 128,
                             n * 512:(n + 1) * 512])
                pb = ps_a.tile([128, 512], FP, tag="psa")
                for kt in range(8):
                    s = kt * 7 + 6
                    nc.tensor.matmul(
                        pb[0:1, :],
                        wproj[:, s * 128: s * 128 + 1],
                        xb[:, kt * 512:(kt + 1) * 512],
                        start=(kt == 0), stop=(kt == 7),
                    )
                nc.scalar.activation(brow[0:1, n * 512:(n + 1) * 512],
                                     pb[0:1, :], AF.Sigmoid)

                for mt in range(6):
                    pt = ps_a.tile([128, 512], FP, tag="psa")
                    for kt in range(8):
                        s = kt * 7 + mt
                        nc.tensor.matmul(
                            pt[:],
                            wproj[:, s * 128:(s + 1) * 128],
                            xb[:, kt * 512:(kt + 1) * 512],
                            start=(kt == 0), stop=(kt == 7),
                        )
                    dst = qkvb[:, mt * (PAD + TOK) + PAD + n * 512:
                               mt * (PAD + TOK) + PAD + (n + 1) * 512]
                    if (mt * NC4 + n) % 2 == 0:
                        nc.scalar.copy(dst, pt[:])
                    else:
                        nc.vector.tensor_copy(dst, pt[:])

            for ci in range(NCH):
                ptt = ps_t.tile([128, 128], BF, tag="ptt")
                nc.tensor.transpose(
                    ptt[:, 0:1], brow[0:1, ci * 128:(ci + 1) * 128],
                    ident[0:1, 0:1])
                nc.scalar.copy(beta_tm[:, ci:ci + 1], ptt[:, 0:1])
            for n in range(NC4):
                nc.gpsimd.partition_broadcast(
                    bb[:, n * 512:(n + 1) * 512], brow[0:1, n * 512:(n + 1) * 512])

            for t in range(6):  # q0 q1 k0 k1 v0 v1
                for n in range(NC4):
                    pt = ps_a.tile([128, 512], FP, tag="psa")
                    base = t * (PAD + TOK) + PAD + n * 512
                    for j in range(CONV_K):
                        nc.tensor.matmul(
                            pt[:],
                            cdiag[:, (t * CONV_K + j) * 128:
                                  (t * CONV_K + j + 1) * 128],
                            qkvb[:, base - j: base - j + 512],
                            start=(j == 0), stop=(j == CONV_K - 1),
                        )
                    if t < 2:
                        dst = q_cm[:, t * TOK + n * 512: t * TOK + (n + 1) * 512]
                    elif t < 4:
                        dst = k_cm[:, (t - 2) * TOK + n * 512:
                                   (t - 2) * TOK + (n + 1) * 512]
                    else:
                        dst = v_cm[:, (t - 4) * (PAD + TOK) + PAD + n * 512:
                                   (t - 4) * (PAD + TOK) + PAD + (n + 1) * 512]
                    nc.scalar.activation(dst, pt[:], AF.Silu)

        # ================= Stage 3: l2norm of q, k; kb =================
        for t_cm in (q_cm, k_cm):
            pn = ps_a.tile([128, 512], FP, tag="psa")
            first = True
            for kt in range(2):
                sq = scr.tile([128, TOK], BF, tag="s4", bufs=2)
                nc.vector.tensor_mul(sq[:],
                                     t_cm[:, kt * TOK:(kt + 1) * TOK],
                                     t_cm[:, kt * TOK:(kt + 1) * TOK])
                for n in range(NC4):
                    nc.tensor.matmul(
                        pn[0:4, :], ones4[:, n * 4:(n + 1) * 4],
                        sq[:, n * 512:(n + 1) * 512],
                        start=first, stop=(kt == 1 and n == NC4 - 1))
                    first = False
            rn = scr.tile([4, 512], FP, tag="s1", bufs=2)
            nc.scalar.activation(rn[:], pn[0:4, :], AF.Sqrt, bias=1e-6)
            nc.vector.reciprocal(rn[:], rn[:])
            rnb = scr.tile([4, 512], BF, tag="s1", bufs=2)
            nc.vector.tensor_copy(rnb[:], rn[:])
            rnrow = scr.tile([1, TOK], BF, tag="row", bufs=1)
            nc.sync.dma_start(rnrow[0:1, :], rnb[:])
            nb = scr.tile([128, TOK], BF, tag="s4", bufs=2)
            for n in range(NC4):
                nc.gpsimd.partition_broadcast(
                    nb[:, n * 512:(n + 1) * 512], rnrow[0:1, n * 512:(n + 1) * 512])
            for kt in range(2):
                nc.vector.tensor_mul(t_cm[:, kt * TOK:(kt + 1) * TOK],
                                     t_cm[:, kt * TOK:(kt + 1) * TOK], nb[:])
        for kt in range(2):
            nc.vector.tensor_mul(kb_cm[:, kt * TOK:(kt + 1) * TOK],
                                 k_cm[:, kt * TOK:(kt + 1) * TOK], bb[:])

        # ================= Stage 4: token-major transposes =================
        mid2 = tc.tile_pool(name="mid2", bufs=1)
        midp2 = mid2.__enter__()
        k_tm = midp2.tile([128, NCH * DK], BF)
        kb_tm = midp2.tile([128, NCH * DK], BF)
        vb_tm = midp2.tile([128, NCH * DV], BF)
        for ci in range(NCH):
            bcol = beta_tm[:, ci:ci + 1]
            for kt in range(2):
                ptt = ps_t.tile([128, 128], BF, tag="ptt")
                nc.tensor.transpose(
                    ptt[:],
                    k_cm[:, kt * TOK + ci * 128: kt * TOK + (ci + 1) * 128],
                    ident[:])
                nc.scalar.copy(
                    k_tm[:, ci * DK + kt * 128: ci * DK + (kt + 1) * 128], ptt[:])
                nc.vector.tensor_scalar_mul(
                    kb_tm[:, ci * DK + kt * 128: ci * DK + (kt + 1) * 128],
                    ptt[:], bcol)
                ptv = ps_t.tile([128, 128], BF, tag="ptt")
                nc.tensor.transpose(
                    ptv[:],
                    v_cm[:, kt * (PAD + TOK) + PAD + ci * 128:
                         kt * (PAD + TOK) + PAD + (ci + 1) * 128],
                    ident[:])
                nc.scalar.activation(
                    vb_tm[:, ci * DV + kt * 128: ci * DV + (kt + 1) * 128],
                    ptv[:], AF.Copy, scale=bcol)

        # ================= Stage 5: delta-rule chunk pre =================
        u_tm = midp2.tile([128, NCH * DV], BF)
        w_cmt = midp2.tile([128, 2 * TOK], BF)
        attn_t = midp2.tile([128, NCH * 128], BF)

        for ci in range(NCH):
            pA = ps_b.tile([128, 256], FP, tag="psb")
            pAt = ps_b.tile([128, 256], FP, tag="psb")
            for kt in range(2):
                sl_k = k_cm[:, kt * TOK + ci * 128: kt * TOK + (ci + 1) * 128]
                sl_kb = kb_cm[:, kt * TOK + ci * 128: kt * TOK + (ci + 1) * 128]
                nc.tensor.matmul(pA[:, 0:128], sl_kb, sl_k,
                                 start=(kt == 0), stop=(kt == 1))
                nc.tensor.matmul(pAt[:, 0:128], sl_k, sl_kb,
                                 start=(kt == 0), stop=(kt == 1))
            Pv = scr.tile([128, 128], BF, tag="P")
            Pt = scr.tile([128, 128], BF, tag="Pt")
            nc.vector.tensor_mul(Pv[:], pA[:, 0:128], msl[:])
            nc.vector.tensor_mul(Pt[:], pAt[:, 0:128], msu[:])
            Tt = scr.tile([128, 128], BF, tag="Tt")
            nc.vector.tensor_sub(Tt[:], ident[:], Pt[:])

            pq = ps_b.tile([128, 256], FP, tag="psb")
            for kt in range(2):
                nc.tensor.matmul(
                    pq[:, 0:128],
                    k_cm[:, kt * TOK + ci * 128: kt * TOK + (ci + 1) * 128],
                    q_cm[:, kt * TOK + ci * 128: kt * TOK + (ci + 1) * 128],
                    start=(kt == 0), stop=(kt == 1))
            nc.vector.tensor_mul(attn_t[:, ci * 128:(ci + 1) * 128],
                                 pq[:, 0:128], msud[:])

            for lvl in range(6):
                psq = ps_b.tile([128, 256], FP, tag="psb")
                nc.tensor.matmul(psq[:, 0:128], Pt[:], Pv[:], start=True, stop=True)
                Pn = scr.tile([128, 128], BF, tag="P")
                nc.scalar.copy(Pn[:], psq[:, 0:128])
                if lvl < 5:
                    psq2 = ps_b.tile([128, 256], FP, tag="psb")
                    nc.tensor.matmul(psq2[:, 0:128], Pv[:], Pt[:],
                                     start=True, stop=True)
                    Ptn = scr.tile([128, 128], BF, tag="Pt")
                    nc.scalar.copy(Ptn[:], psq2[:, 0:128])
                else:
                    Ptn = Pt
                pprod = ps_b.tile([128, 256], FP, tag="psb")
                nc.tensor.matmul(pprod[:, 0:128], Pn[:], Tt[:],
                                 start=True, stop=False)
                nc.tensor.matmul(pprod[:, 0:128], ident[:], Tt[:],
                                 start=False, stop=True)
                Ttn = scr.tile([128, 128], BF, tag="Tt")
                if lvl % 2 == 0:
                    nc.vector.tensor_copy(Ttn[:], pprod[:, 0:128])
                else:
                    nc.scalar.copy(Ttn[:], pprod[:, 0:128])
                Pv, Pt, Tt = Pn, Ptn, Ttn

            pu = ps_b.tile([128, 256], FP, tag="psb")
            nc.tensor.matmul(pu[:], Tt[:], vb_tm[:, ci * DV:(ci + 1) * DV],
                             start=True, stop=True)
            nc.scalar.copy(u_tm[:, ci * DV:(ci + 1) * DV], pu[:])
            for kt in range(2):
                pw = ps_b.tile([128, 256], FP, tag="psb")
                nc.tensor.matmul(
                    pw[:, 0:128],
                    kb_tm[:, ci * DK + kt * 128: ci * DK + (kt + 1) * 128],
                    Tt[:], start=True, stop=True)
                nc.vector.tensor_copy(
                    w_cmt[:, kt * TOK + ci * 128: kt * TOK + (ci + 1) * 128],
                    pw[:, 0:128])

        # ================= Stage 6: FIR convs =================
        with tc.tile_pool(name="fir", bufs=1) as firp:
            for kt in range(2):
                fsd = firp.tile([128, FIR_S * 128], BF, tag="fsd")
                for j in range(FIR_S):
                    build_diag(fsd[:, j * 128:(j + 1) * 128],
                               CP_FSTAPS + kt * FIR_S + j)
                fld = firp.tile([128, FIR_L * 128], BF, tag="fld")
                for j in range(FIR_L):
                    build_diag(fld[:, j * 128:(j + 1) * 128],
                               CP_FLTAPS + kt * FIR_L + j)
                vbase = kt * (PAD + TOK) + PAD
                for n in range(NC4):
                    pt = ps_a.tile([128, 512], FP, tag="psa")
                    for j in range(FIR_S):
                        nc.tensor.matmul(
                            pt[:], fsd[:, j * 128:(j + 1) * 128],
                            v_cm[:, vbase + n * 512 - j: vbase + (n + 1) * 512 - j],
                            start=(j == 0), stop=(j == FIR_S - 1))
                    nc.scalar.copy(
                        fir_s[:, kt * TOK + n * 512: kt * TOK + (n + 1) * 512],
                        pt[:])
                    pt2 = ps_a.tile([128, 512], FP, tag="psa")
                    for j in range(FIR_L):
                        nc.tensor.matmul(
                            pt2[:], fld[:, j * 128:(j + 1) * 128],
                            v_cm[:, vbase + n * 512 - j: vbase + (n + 1) * 512 - j],
                            start=(j == 0), stop=(j == FIR_L - 1))
                    nc.scalar.copy(
                        fir_l[:, kt * TOK + n * 512: kt * TOK + (n + 1) * 512],
                        pt2[:])

        # ================= Stage 7: serial scan =================
        for ci in range(NCH):
            pu2 = ps_b.tile([128, 256], FP, tag="psb")
            for kt in range(2):
                nc.tensor.matmul(
                    pu2[:],
                    w_cmt[:, kt * TOK + ci * 128: kt * TOK + (ci + 1) * 128],
                    S_bf[:, kt * DV:(kt + 1) * DV],
                    start=(kt == 0), stop=(kt == 1))
            u2 = scr.tile([128, 256], BF, tag="u2")
            nc.vector.tensor_sub(u2[:], u_tm[:, ci * DV:(ci + 1) * DV], pu2[:])
            po = ps_b.tile([128, 256], FP, tag="psb")
            for kt in range(2):
                nc.tensor.matmul(
                    po[:],
                    q_cm[:, kt * TOK + ci * 128: kt * TOK + (ci + 1) * 128],
                    S_bf[:, kt * DV:(kt + 1) * DV],
                    start=(kt == 0), stop=False)
            nc.tensor.matmul(po[:], attn_t[:, ci * 128:(ci + 1) * 128], u2[:],
                             start=False, stop=True)
            nc.scalar.copy(delta_tm[:, ci * DV:(ci + 1) * DV], po[:])
            pS = ps_b.tile([128, 256], FP, tag="psb")
            nc.tensor.matmul(pS[:], k_tm[:, ci * DK: ci * DK + 128], u2[:],
                             start=True, stop=True)
            pS2 = ps_b.tile([128, 256], FP, tag="psb")
            nc.tensor.matmul(pS2[:], k_tm[:, ci * DK + 128: ci * DK + 256], u2[:],
                             start=True, stop=True)
            nc.vector.tensor_add(S_bf[:, 0:DV], S_bf[:, 0:DV], pS[:])
            nc.vector.tensor_add(S_bf[:, DV:2 * DV], S_bf[:, DV:2 * DV], pS2[:])

        for ci in range(NCH):
            for kt in range(2):
                ptt = ps_t.tile([128, 128], BF, tag="ptt")
                nc.tensor.transpose(
                    ptt[:],
                    delta_tm[:, ci * DV + kt * 128: ci * DV + (kt + 1) * 128],
                    ident[:])
                nc.scalar.copy(
                    delta_cm[:, kt * TOK + ci * 128: kt * TOK + (ci + 1) * 128],
                    ptt[:])

        # ================= Stage 8: branch stats =================
        st_f32 = scr.tile([16, 512], FP, tag="s1", bufs=2)
        branches = ((0, fir_s, 0, TOK), (1, fir_l, 0, TOK),
                    (2, delta_cm, 0, TOK), (3, v_cm, PAD, PAD + TOK))
        for n in range(NC4):
            pst = ps_a.tile([128, 512], FP, tag="psa")
            for bi, tns, off, stride in branches:
                for kt in range(2):
                    src = tns[:, kt * stride + off + n * 512:
                              kt * stride + off + (n + 1) * 512]
                    nc.tensor.matmul(
                        pst[0:48, :],
                        sumsel[:, (bi * 2) * 48:(bi * 2 + 1) * 48], src,
                        start=(bi == 0 and kt == 0), stop=False)
                    sqt = scr.tile([128, 512], BF, tag="s1", bufs=2)
                    nc.vector.tensor_mul(sqt[:], src, src)
                    nc.tensor.matmul(
                        pst[0:48, :],
                        sumsel[:, (bi * 2 + 1) * 48:(bi * 2 + 2) * 48], sqt[:],
                        start=False, stop=(bi == 3 and kt == 1))
            nc.scalar.activation(stats32[0:16, n * 512:(n + 1) * 512],
                                 pst[0:16, :], AF.Copy, scale=1.0 / DV)
            msq = scr.tile([16, 512], FP, tag="s1", bufs=2)
            nc.vector.tensor_mul(msq[:], stats32[0:16, n * 512:(n + 1) * 512],
                                 stats32[0:16, n * 512:(n + 1) * 512])
            nc.vector.scalar_tensor_tensor(
                st_f32[:], pst[32:48, :], 1.0 / DV, msq[:],
                op0=OP.mult, op1=OP.subtract)
            nc.vector.tensor_scalar_max(st_f32[:], st_f32[:], 1e-6)
            nc.scalar.activation(stats32[32:48, n * 512:(n + 1) * 512],
                                 st_f32[:], AF.Sqrt)

        mid2.__exit__(None, None, None)
        mid.__exit__(None, None, None)

        # ============ Stage 9: stats AllReduce (all tokens) ========
        st_in = dram.tile([32, TOK], BF)
        st_out = dram.tile([32, TOK], BF)
        nc.sync.dma_start(st_in[0:16, :], stats32[0:16, :])
        nc.sync.dma_start(st_in[16:32, :], stats32[32:48, :])
        nc.gpsimd.collective_compute(
            "AllReduce", mybir.AluOpType.add, replica_groups=rg4,
            ins=[st_in[:]], outs=[st_out[:]])
        nc.sync.dma_start(stats_sb[0:32, :], st_out[:])

        # ====== Stage 10: gate MLP (4 of 16 hidden tiles, all tokens) ========
        with tc.tile_pool(name="tail", bufs=1) as tail, \
             tc.tile_pool(name="w1p", bufs=3) as w1p:
            w1t4 = tail.tile([128, 4 * 1152], BF)
            for mt2 in range(4):
                nc.sync.dma_start(w1t4[:, mt2 * 1152:(mt2 + 1) * 1152],
                                  g_w1[mt2 * 128:(mt2 + 1) * 128, :])
            h1 = tail.tile([128, 4 * TOK], BF)
            for n in range(NC4):
                xb2 = w1p.tile([128, 8 * 512], BF, tag="xb2", bufs=2)
                for kt in range(8):
                    nc.sync.dma_start(
                        xb2[:, kt * 512:(kt + 1) * 512],
                        g_xT[kt * 128:(kt + 1) * 128, n * 512:(n + 1) * 512])
                for mt2 in range(4):
                    pt = ps_a.tile([128, 512], FP, tag="psa")
                    for kt in range(9):
                        rhs = (xb2[:, kt * 512:(kt + 1) * 512] if kt < 8
                               else stats_sb[:, n * 512:(n + 1) * 512])
                        nc.tensor.matmul(
                            pt[:], w1t4[:, mt2 * 1152 + kt * 128:
                                        mt2 * 1152 + (kt + 1) * 128],
                            rhs, start=(kt == 0), stop=(kt == 8))
                    nc.scalar.activation(
                        h1[:, mt2 * TOK + n * 512: mt2 * TOK + (n + 1) * 512],
                        pt[:], AF.Gelu, bias=b1c[:, mt2:mt2 + 1])

            pls = tail.tile([16, 4 * 512], FP)
            for n in range(NC4):
                pl = ps_a.tile([128, 512], FP, tag="psa")
                for mt2 in range(4):
                    nc.tensor.matmul(
                        pl[0:16, :], w2my[:, mt2 * 16:(mt2 + 1) * 16],
                        h1[:, mt2 * TOK + n * 512: mt2 * TOK + (n + 1) * 512],
                        start=(mt2 == 0), stop=(mt2 == 3))
                nc.scalar.copy(pls[:, n * 512:(n + 1) * 512], pl[0:16, :])

            pl_in = dram.tile([64, 512], FP)
            pl_out = dram.tile([16, 512], FP)
            for n in range(NC4):
                nc.sync.dma_start(pl_in[n * 16:(n + 1) * 16, :],
                                  pls[:, n * 512:(n + 1) * 512])
            nc.gpsimd.collective_compute(
                "ReduceScatter", mybir.AluOpType.add, replica_groups=rg4,
                ins=[pl_in[:]], outs=[pl_out[:]])
            plq = tail.tile([16, 512], FP)
            nc.sync.dma_start(plq[:], pl_out[:])

            expt = tail.tile([16, 512], BF)
            nc.scalar.activation(expt[:], plq[:], AF.Exp, bias=b2c[:, 0:1])
            pg = ps_b.tile([128, 256], FP, tag="psb")
            nc.tensor.matmul(pg[0:16, :], bsum[:], expt[:, 0:256],
                             start=True, stop=True)
            pg2 = ps_b.tile([128, 256], FP, tag="psb")
            nc.tensor.matmul(pg2[0:16, :], bsum[:], expt[:, 256:512],
                             start=True, stop=True)
            gsum = tail.tile([16, 512], FP)
            nc.scalar.copy(gsum[:, 0:256], pg[0:16, :])
            nc.scalar.copy(gsum[:, 256:512], pg2[0:16, :])
            nc.vector.reciprocal(gsum[:], gsum[:])
            wg = tail.tile([16, 512], BF)
            nc.vector.tensor_mul(wg[:], expt[:], gsum[:])

            # ============ Stage 11: gate AllGather + extraction ============
            wg_in = dram.tile([16, 512], BF)
            wg_out = dram.tile([64, 512], BF)
            nc.sync.dma_start(wg_in[:], wg[:])
            nc.gpsimd.collective_compute(
                "AllGather", mybir.AluOpType.bypass, replica_groups=rg4,
                ins=[wg_in[:]], outs=[wg_out[:]])
            wrow = tail.tile([1, 4 * TOK], BF)
            for g in range(4):
                gt = w1p.tile([16, 512], BF, tag="gath")
                nc.sync.dma_start(gt[:], wg_out[g * 16:(g + 1) * 16, :])
                pw4 = ps_b.tile([128, 256], FP, tag="psb")
                nc.tensor.matmul(pw4[:], hsel[:], gt[:, 0:256],
                                 start=True, stop=True)
                pw42 = ps_b.tile([128, 256], FP, tag="psb")
                nc.tensor.matmul(pw42[:], hsel[:], gt[:, 256:512],
                                 start=True, stop=True)
                for r in range(4):
                    nc.scalar.copy(
                        wrow[0:1, r * TOK + g * 512: r * TOK + g * 512 + 256],
                        pw4[32 * r:32 * r + 1, :])
                    nc.scalar.copy(
                        wrow[0:1, r * TOK + g * 512 + 256: r * TOK + (g + 1) * 512],
                        pw42[32 * r:32 * r + 1, :])

            wb4 = tail.tile([128, 4 * TOK], BF)
            for j in range(4):
                for n in range(NC4):
                    nc.gpsimd.partition_broadcast(
                        wb4[:, j * TOK + n * 512: j * TOK + (n + 1) * 512],
                        wrow[0:1, j * TOK + n * 512: j * TOK + (n + 1) * 512])

            # ============ Stage 12: mix + RMSNorm ============
            for kt in range(2):
                t1 = o_mix[:, kt * TOK:(kt + 1) * TOK]
                t2 = scr.tile([128, TOK], BF, tag="s4", bufs=2)
                nc.vector.tensor_mul(t1, wb4[:, 0:TOK],
                                     fir_s[:, kt * TOK:(kt + 1) * TOK])
                nc.vector.tensor_mul(t2[:], wb4[:, TOK:2 * TOK],
                                     fir_l[:, kt * TOK:(kt + 1) * TOK])
                nc.vector.tensor_add(t1, t1, t2[:])
                nc.vector.tensor_mul(t2[:], wb4[:, 2 * TOK:3 * TOK],
                                     delta_cm[:, kt * TOK:(kt + 1) * TOK])
                nc.vector.tensor_add(t1, t1, t2[:])
                nc.vector.tensor_mul(
                    t2[:], wb4[:, 3 * TOK:4 * TOK],
                    v_cm[:, kt * (PAD + TOK) + PAD: kt * (PAD + TOK) + PAD + TOK])
                nc.vector.tensor_add(t1, t1, t2[:])

            prms = ps_a.tile([128, 512], FP, tag="psa")
            first = True
            for n in range(NC4):
                for kt in range(2):
                    sqm = scr.tile([128, 512], BF, tag="s1", bufs=2)
                    src = o_mix[:, kt * TOK + n * 512: kt * TOK + (n + 1) * 512]
                    nc.vector.tensor_mul(sqm[:], src, src)
                    nc.tensor.matmul(prms[0:4, :], ones4[:, n * 4:(n + 1) * 4],
                                     sqm[:], start=first,
                                     stop=(n == NC4 - 1 and kt == 1))
                    first = False
            rms = scr.tile([4, 512], FP, tag="s1", bufs=2)
            nc.scalar.activation(rms[:], prms[0:4, :], AF.Sqrt,
                                 scale=1.0 / DV, bias=1e-5)
            nc.vector.reciprocal(rms[:], rms[:])
            rmsb = scr.tile([4, 512], BF, tag="s1", bufs=2)
            nc.vector.tensor_copy(rmsb[:], rms[:])
            rmsrow = scr.tile([1, TOK], BF, tag="row", bufs=1)
            nc.sync.dma_start(rmsrow[0:1, :], rmsb[:])
            rb = scr.tile([128, TOK], BF, tag="s4", bufs=2)
            for n in range(NC4):
                nc.gpsimd.partition_broadcast(
                    rb[:, n * 512:(n + 1) * 512], rmsrow[0:1, n * 512:(n + 1) * 512])
            for kt in range(2):
                nc.vector.scalar_tensor_tensor(
                    o_mix[:, kt * TOK:(kt + 1) * TOK],
                    o_mix[:, kt * TOK:(kt + 1) * TOK],
                    onwc[:, kt:kt + 1], rb[:], op0=OP.mult, op1=OP.mult)

            # ===== Stage 13+14: partial output projection + ReduceScatter ====
            rs_in = dram.tile([2048, 1024], BF)
            rs_out = dram.tile([512, 1024], BF)
            for mt in range(16):
                ost = w1p.tile([128, 1024], BF, tag="ost")
                for n2 in range(2):
                    pt = ps_a.tile([128, 512], FP, tag="psa")
                    for kt in range(2):
                        nc.tensor.matmul(
                            pt[:],
                            o_mix[:, kt * TOK + mt * 128: kt * TOK + (mt + 1) * 128],
                            wos[:, kt * 1024 + n2 * 512: kt * 1024 + (n2 + 1) * 512],
                            start=(kt == 0), stop=(kt == 1))
                    nc.scalar.copy(ost[:, n2 * 512:(n2 + 1) * 512], pt[:])
                nc.sync.dma_start(rs_in[mt * 128:(mt + 1) * 128, :], ost[:])
            nc.gpsimd.collective_compute(
                "ReduceScatter", mybir.AluOpType.add, replica_groups=rg4,
                ins=[rs_in[:]], outs=[rs_out[:]])
            nc.sync.dma_start(d_out.ap(), rs_out[:])

    nc.compile()
    return nc


def _host_prep(inputs):
    """Build the global (concat-over-cores along axis 0) input arrays."""
    x = np.asarray(inputs["hidden_states"], F32NP)
    Wq = np.asarray(inputs["Wq"], F32NP)
    Wk = np.asarray(inputs["Wk"], F32NP)
    Wv = np.asarray(inputs["Wv"], F32NP)
    Wb = np.asarray(inputs["Wb"], F32NP)
    cqw = np.asarray(inputs["conv_q_w"], F32NP)
    ckw = np.asarray(inputs["conv_k_w"], F32NP)
    cvw = np.asarray(inputs["conv_v_w"], F32NP)
    fsw = np.asarray(inputs["fir_short_w"], F32NP).reshape(NH * DV, FIR_S)
    flw = np.asarray(inputs["fir_long_w"], F32NP).reshape(NH * DV, FIR_L)
    w1 = np.asarray(inputs["mlp_w1"], F32NP)
    b1 = np.asarray(inputs["mlp_b1"], F32NP)
    w2 = np.asarray(inputs["mlp_w2"], F32NP)
    b2 = np.asarray(inputs["mlp_b2"], F32NP)
    glt = np.asarray(inputs["gate_log_temp"], np.float64)
    onw = np.asarray(inputs["o_norm_w"], F32NP)
    Wo = np.asarray(inputs["Wo"], F32NP)

    temp = (np.log1p(np.exp(glt)) + 1e-4).astype(F32NP)
    tcol = np.repeat(temp, 4)
    w2f = (w2 / tcol[None, :]).astype(F32NP)
    b2f = (b2 / tcol).astype(F32NP)

    # x channel-quarter slices, already in per-core concat order
    xq = np.ascontiguousarray(np.transpose(x, (0, 2, 1))) \
        .reshape(8 * 256, TOK).astype(BF16NP)

    # w1 with stats rows permuted to device layout + pad to 1152
    w1p = np.zeros((1152, MLP_H), F32NP)
    w1p[0:1024] = w1[0:1024]
    bi = np.arange(4)[:, None]
    hh = np.arange(4)[None, :]
    w1p[(1024 + bi * 4 + hh).ravel()] = w1[(1024 + bi * 8 + hh).ravel()]
    w1p[(1040 + bi * 4 + hh).ravel()] = w1[(1028 + bi * 8 + hh).ravel()]
    # w1s block (mt, kt) = w1p block (kt, mt)
    w1s = w1p.reshape(9, 128, 16, 128).transpose(2, 1, 0, 3) \
        .reshape(16 * 128, 9 * 128).astype(BF16NP)
    # core (b, g) ships w1s rows [g*512 + b*256 : +256]
    w1q = np.ascontiguousarray(
        w1s.reshape(4, 2, 256, 1152).transpose(1, 0, 2, 3).reshape(2048, 1152))

    idx = np.arange(128)
    ident = np.zeros((128, 128), F32NP)
    ident[idx, idx] = 1
    ii = idx[:, None]
    jj = idx[None, :]
    msl = (ii > jj).astype(F32NP)
    msu = (ii < jj).astype(F32NP)
    msud = (ii <= jj).astype(F32NP)
    ones4 = np.zeros((128, 16), F32NP)
    for n in range(4):
        ones4[:, n * 4 + n] = 1
    bsum = np.zeros((16, 16), F32NP)
    for kk in range(16):
        for mm in range(16):
            if kk // 4 == mm // 4:
                bsum[kk, mm] = 1

    whalf = np.zeros((2, 4, 64, W_WH), BF16NP)
    for h in range(4):
        hsl = slice(h * 256, (h + 1) * 256)
        wh_h = np.zeros((128, W_WH), F32NP)
        wproj_full = np.zeros((1024, 896), F32NP)
        wproj_full[:, 0:256] = Wq[:, hsl]
        wproj_full[:, 256:512] = Wk[:, hsl]
        wproj_full[:, 512:768] = Wv[:, hsl]
        wproj_full[:, 768] = Wb[:, h]
        wh_h[:, 0:W_WPROJ] = wproj_full.reshape(8, 128, 7, 128) \
            .transpose(1, 0, 2, 3).reshape(128, W_WPROJ)
        wh_h[:, W_WPROJ:W_WPROJ + W_WOS] = Wo[hsl].reshape(2, 128, 1024) \
            .transpose(1, 0, 2).reshape(128, 2048)

        cp = wh_h[:, W_WPROJ + W_WOS:]
        cp[:, CP_IDENT:CP_IDENT + 128] = ident
        cp[:, CP_MSL:CP_MSL + 128] = msl
        cp[:, CP_MSU:CP_MSU + 128] = msu
        cp[:, CP_MSUD:CP_MSUD + 128] = msud
        cp[:, CP_ONES4:CP_ONES4 + 16] = ones4
        for bi2 in range(4):
            cp[:, CP_SUMSEL + (bi2 * 2) * 48 + bi2 * 4 + h] = 1
            cp[:, CP_SUMSEL + (bi2 * 2 + 1) * 48 + 32 + bi2 * 4 + h] = 1
        for mt2 in range(4):
            cp[:, CP_W2 + mt2 * 16: CP_W2 + (mt2 + 1) * 16] = \
                w2f[(4 * h + mt2) * 128:(4 * h + mt2 + 1) * 128, :]
        cp[0:16, CP_BSUM:CP_BSUM + 16] = bsum
        for r in range(4):
            cp[4 * h + r, CP_HSEL + 32 * r] = 1
        for ti, cw in enumerate((cqw[hsl], ckw[hsl], cvw[hsl])):
            for t2 in range(2):
                for j in range(CONV_K):
                    cp[:, CP_CTAPS + (ti * 2 + t2) * CONV_K + (CONV_K - 1 - j)] \
                        = cw[t2 * 128:(t2 + 1) * 128, j]
        for kt in range(2):
            for j in range(FIR_S):
                cp[:, CP_FSTAPS + kt * FIR_S + (FIR_S - 1 - j)] = \
                    fsw[hsl][kt * 128:(kt + 1) * 128, j]
            for j in range(FIR_L):
                cp[:, CP_FLTAPS + kt * FIR_L + (FIR_L - 1 - j)] = \
                    flw[hsl][kt * 128:(kt + 1) * 128, j]
        whb = wh_h.astype(BF16NP)
        whalf[0, h] = whb[0:64]
        whalf[1, h] = whb[64:128]
    whalf = whalf.reshape(8 * 64, W_WH)

    fp = np.zeros((4, 128, 7), F32NP)
    for g in range(4):
        for j in range(4):
            fp[g, :, j] = b1[(4 * g + j) * 128:(4 * g + j + 1) * 128]
    fp[:, 0:16, 4] = b2f
    fp[:, :, 5] = onw[0:128]
    fp[:, :, 6] = onw[128:256]
    fpack = np.concatenate([fp, fp], 0).reshape(8, 128 * 7).astype(BF16NP)

    blob = np.empty((8, NBLOB), BF16NP)
    blob[:, O_XQ:O_WH] = xq.reshape(8, -1)
    blob[:, O_WH:O_W1] = whalf.reshape(8, -1)
    blob[:, O_W1:O_FP] = w1q.reshape(8, -1)
    blob[:, O_FP:] = fpack
    return {"blob": blob}


def _get_nc():
    if "nc" not in _CACHE:
        _CACHE["nc"] = _build_program()
    return _CACHE["nc"]


def _make_sharding():
    import jax
    from jax.sharding import Mesh, NamedSharding, PartitionSpec

    devices = jax.devices()[:8]
    mesh = Mesh(np.asarray(devices), ("core",))
    return mesh, NamedSharding(mesh, PartitionSpec("core"))


def _make_compiled(nc):
    """AOT-compile the sharded bass_exec callable (and an on-device zeros
    producer for the donated output buffer)."""
    import jax
    import jax.numpy as jnp
    import concourse.mybir as mybir
    from jax.sharding import PartitionSpec
    from jax.experimental.shard_map import shard_map
    from concourse import bass2jax
    from concourse.bass2jax import _bass_exec_p, partition_id_tensor

    bass2jax.install_neuronx_cc_hook()
    partition_name = (nc.partition_id_tensor.name
                      if nc.partition_id_tensor else None)
    in_names, out_names, out_avals = [], [], []
    for alloc in nc.m.functions[0].allocations:
        if not isinstance(alloc, mybir.MemoryLocationSet):
            continue
        name = alloc.memorylocations[0].name
        if alloc.kind == "ExternalInput":
            if name != partition_name:
                in_names.append(name)
        elif alloc.kind == "ExternalOutput":
            out_names.append(name)
            out_avals.append(jax.core.ShapedArray(
                tuple(alloc.tensor_shape), mybir.dt.np(alloc.dtype)))
    n_params = len(in_names)
    all_in_names = list(in_names) + out_names
    if partition_name is not None:
        all_in_names.append(partition_name)

    def _body(*args):
        operands = list(args)
        if partition_name is not None:
            operands.append(partition_id_tensor())
        return tuple(_bass_exec_p.bind(
            *operands, out_avals=tuple(out_avals), in_names=tuple(all_in_names),
            out_names=tuple(out_names), lowering_input_output_aliases=(),
            sim_require_finite=True, sim_require_nnan=True, nc=nc))

    mesh, sh = _make_sharding()
    donate = tuple(range(n_params, n_params + len(out_names)))
    sharded = jax.jit(
        shard_map(_body, mesh=mesh,
                  in_specs=(PartitionSpec("core"),) * (n_params + len(out_names)),
                  out_specs=(PartitionSpec("core"),) * len(out_names),
                  check_rep=False),
        donate_argnums=donate, keep_unused=True)

    def g_spec(name):
        for alloc in nc.m.functions[0].allocations:
            if (isinstance(alloc, mybir.MemoryLocationSet)
                    and alloc.memorylocations[0].name == name):
                shp = tuple(alloc.tensor_shape)
                return jax.ShapeDtypeStruct(
                    (8 * shp[0],) + shp[1:], mybir.dt.np(alloc.dtype),
                    sharding=sh)
        raise KeyError(name)

    specs = [g_spec(n) for n in in_names] + [g_spec(n) for n in out_names]
    compiled = sharded.lower(*specs).compile()
    zeros = jax.jit(
        lambda: jnp.zeros((8 * TSL, H), jnp.bfloat16),
        out_shardings=sh).lower().compile()
    return {"compiled": compiled, "zeros": zeros,
            "in_names": in_names, "out_names": out_names}


_INIT = {}


def _bg_init():
    try:
        import jax

        try:
            jax.config.update("jax_compilation_cache_dir", "/root/.jax_cache")
            jax.config.update("jax_persistent_cache_min_entry_size_bytes", -1)
            jax.config.update("jax_persistent_cache_min_compile_time_secs", 0)
        except Exception:
            pass
        nc = _get_nc()
        _INIT.update(_make_compiled(nc))
    except BaseException as e:  # noqa: BLE001 - reraised in kernel()
        _INIT["err"] = e


_BG = threading.Thread(target=_bg_init, daemon=True)
_BG.start()


def kernel(**inputs):
    globs = _host_prep(inputs)

    import jax

    _mesh, sh = _make_sharding()
    put = {k: jax.device_put(v, sh) for k, v in globs.items()}
    _BG.join()
    if "err" in _INIT:
        raise _INIT["err"]

    args = [put[n] for n in _INIT["in_names"]] + [_INIT["zeros"]()]
    out_arrs = _INIT["compiled"](*args)
    o = np.asarray(out_arrs[0]).astype(F32NP).reshape(8, TSL, H)
    full = np.empty((B, L, H), F32NP)
    for c in range(8):
        full[c // 4, (c % 4) * TSL:(c % 4 + 1) * TSL] = o[c]
    return full


def run_traced(inputs, trace=True):
    """Dev helper: run via run_bass_kernel_spmd to capture a profile."""
    from concourse.bass_utils import run_bass_kernel_spmd

    nc = _get_nc()
    globs = _host_prep(inputs)
    in_maps = [
        {"blob": np.ascontiguousarray(globs["blob"][c:c + 1])}
        for c in range(8)
    ]
    res = run_bass_kernel_spmd(nc, in_maps, core_ids=list(range(8)), trace=trace)
    out = np.zeros((B, L, H), F32NP)
    for c in range(8):
        out[c // 4, (c % 4) * TSL:(c % 4 + 1) * TSL] = \
            np.asarray(res.results[c]["out"], F32NP)
    return out, res


# revision 14
# speedup vs baseline: 1.0449x; 1.0449x over previous
"""DeltaNet block as a Bass/Tile SPMD kernel on 8 TRN2 NeuronCores.

Sharding: one (batch, head) pair per core (B=2 x NH=4 = 8 cores).

Host->device traffic is minimized: each core uploads only a 1/4-channel
slice of its batch's x^T, half of its head's packed weights (the batch-pair
core uploads the other half), and a quarter of its group's gate-MLP w1 slice.
On-device AllGathers (4-way for x, pair-wise for weights) reconstruct the
full operands over NeuronLink. Depthwise-conv/FIR diagonal matrices are
built on device from compact tap vectors (ident * tap column).

Per core: q/k/v/beta projections (head slice) + causal convs + silu, l2norm,
chunkwise delta rule (C=128 chunks, product-form unit-triangular inverse),
FIR convs (PE diagonal-matmul), branch stats, a 4-core AllReduce of the
stats rows, the gate MLP (4 of 16 hidden tiles x all tokens, with a
partial-logit ReduceScatter), AllGather of gate weights, channel-major
4-way mix + RMSNorm, the output projection and a ReduceScatter that both
sums heads and splits tokens. Output is returned bf16 and upcast on host.

All matmul operands are bf16 (fp32 PSUM accumulation).
"""

import sys
import threading

import numpy as np

if "/opt/trn_rl_repo" not in sys.path:
    sys.path.insert(0, "/opt/trn_rl_repo")

import ml_dtypes

BF16NP = ml_dtypes.bfloat16
F32NP = np.float32

B, L, H = 2, 2048, 1024
NH, DK, DV = 4, 256, 256
CONV_K, FIR_S, FIR_L = 4, 3, 63
MLP_H = 2 * H
C = 128
NCH = L // C
PAD = 64
TOK = L
TSL = L // 4
NC4 = TOK // 512

# packed-weight column offsets inside cpack
CP_IDENT = 0
CP_MSL = 128
CP_MSU = 256
CP_MSUD = 384
CP_ONES4 = 512
CP_SUMSEL = 528          # 8 blocks x 48
CP_W2 = 912              # 4 blocks x 16
CP_BSUM = 976            # [16,16]
CP_HSEL = 992            # [16,128]
CP_CTAPS = 1120          # 6 tensors x 4 taps
CP_FSTAPS = 1144         # 2 kt x 3 taps
CP_FLTAPS = 1150         # 2 kt x 63 taps
W_CPACK = 1280
W_WPROJ = 7168
W_WOS = 2048
W_WH = W_WPROJ + W_WOS + W_CPACK  # 10496

# single fused per-core input blob (bf16 elements)
O_XQ = 0
O_WH = O_XQ + 256 * TOK          # 524288
O_W1 = O_WH + 64 * W_WH          # 1196032
O_FP = O_W1 + 256 * 1152         # 1490944
NBLOB = O_FP + 128 * 7           # 1491840

_CACHE = {}


def _build_program():
    from contextlib import ExitStack

    import concourse.bacc as bacc
    import concourse.mybir as mybir
    import concourse.tile as tile

    dt = mybir.dt
    BF = dt.bfloat16
    FP = dt.float32
    AF = mybir.ActivationFunctionType
    OP = mybir.AluOpType

    nc = bacc.Bacc("TRN2", target_bir_lowering=False, debug=False, num_devices=8)

    for v in (1e-6, 1e-5):
        t = nc.alloc_sbuf_tensor(f"const-float32-{v}", [128, 1], FP)
        nc.gpsimd.memset(t.ap(), v)
        nc.const_aps.aps[(FP, v)] = t.ap()
    nc.all_engine_barrier()

    d_blob = nc.dram_tensor("blob", [1, NBLOB], BF, kind="ExternalInput")
    d_out = nc.dram_tensor("out", [TSL, H], BF, kind="ExternalOutput")

    rg4 = [[0, 1, 2, 3], [4, 5, 6, 7]]
    rgp = [[0, 4], [1, 5], [2, 6], [3, 7]]

    with tile.TileContext(nc) as tc, ExitStack() as es:
        cst = es.enter_context(tc.tile_pool(name="cst", bufs=1))
        per = es.enter_context(tc.tile_pool(name="per", bufs=1))
        scr = es.enter_context(tc.tile_pool(name="scr", bufs=3))
        ps_a = es.enter_context(tc.tile_pool(name="ps_a", bufs=2, space="PSUM"))
        ps_b = es.enter_context(tc.tile_pool(name="ps_b", bufs=4, space="PSUM"))
        ps_t = es.enter_context(tc.tile_pool(name="ps_t", bufs=2, space="PSUM"))
        dram = es.enter_context(tc.tile_pool(name="dram", bufs=1, space="DRAM"))

        # ============ Stage 0: assemble inputs via on-device AllGather =======
        i_xq = dram.tile([256, TOK], BF)
        i_wh = dram.tile([64, W_WH], BF)
        i_w1 = dram.tile([256, 1152], BF)
        g_xT = dram.tile([1024, TOK], BF)
        g_w = dram.tile([128, W_WH], BF)
        g_w1 = dram.tile([512, 1152], BF)
        nc.sync.dma_start(i_xq[:], d_blob.ap()[0:1, O_XQ:O_WH])
        nc.sync.dma_start(i_wh[:], d_blob.ap()[0:1, O_WH:O_W1])
        nc.sync.dma_start(i_w1[:], d_blob.ap()[0:1, O_W1:O_FP])
        nc.gpsimd.collective_compute(
            "AllGather", mybir.AluOpType.bypass, replica_groups=rg4,
            ins=[i_xq[:]], outs=[g_xT[:]])
        nc.gpsimd.collective_compute(
            "AllGather", mybir.AluOpType.bypass, replica_groups=rgp,
            ins=[i_wh[:]], outs=[g_w[:]])
        nc.gpsimd.collective_compute(
            "AllGather", mybir.AluOpType.bypass, replica_groups=rgp,
            ins=[i_w1[:]], outs=[g_w1[:]])

        cpack = cst.tile([128, W_CPACK], BF)
        nc.sync.dma_start(cpack[:], g_w[:, W_WPROJ + W_WOS: W_WH])
        fpbf = cst.tile([128, 7], BF)
        nc.sync.dma_start(fpbf[:], d_blob.ap()[0:1, O_FP:NBLOB])
        fpack = cst.tile([128, 7], FP)
        nc.vector.tensor_copy(fpack[:], fpbf[:])
        wos = cst.tile([128, W_WOS], BF)
        nc.sync.dma_start(wos[:], g_w[:, W_WPROJ: W_WPROJ + W_WOS])

        ident = cpack[:, CP_IDENT:CP_IDENT + 128]
        msl = cpack[:, CP_MSL:CP_MSL + 128]
        msu = cpack[:, CP_MSU:CP_MSU + 128]
        msud = cpack[:, CP_MSUD:CP_MSUD + 128]
        ones4 = cpack[:, CP_ONES4:CP_ONES4 + 16]
        sumsel = cpack[:, CP_SUMSEL:CP_SUMSEL + 384]
        w2my = cpack[:, CP_W2:CP_W2 + 64]
        bsum = cpack[0:16, CP_BSUM:CP_BSUM + 16]
        hsel = cpack[0:16, CP_HSEL:CP_HSEL + 128]
        b1c = fpack[:, 0:4]
        b2c = fpack[0:16, 4:5]
        onwc = fpack[:, 5:7]

        # fp32 copy of the tap columns (scalar operands must be fp32)
        tapf = cst.tile([128, W_CPACK - CP_CTAPS], FP)
        nc.vector.tensor_copy(tapf[:], cpack[:, CP_CTAPS:W_CPACK])

        def build_diag(dst, col):
            # dst[128,128] = diag(tap column `col` of cpack); alternate engines
            scol = tapf[:, col - CP_CTAPS: col - CP_CTAPS + 1]
            if col % 2 == 0:
                nc.scalar.activation(dst, ident, AF.Copy, scale=scol)
            else:
                nc.vector.tensor_scalar_mul(dst, ident, scol)

        v_cm = per.tile([128, 2 * (PAD + TOK)], BF)
        fir_s = per.tile([128, 2 * TOK], BF)
        fir_l = per.tile([128, 2 * TOK], BF)
        delta_cm = per.tile([128, 2 * TOK], BF)
        brow = per.tile([1, TOK], BF)
        beta_tm = per.tile([128, NCH], FP)
        S_bf = per.tile([128, 2 * DV], BF)
        stats32 = per.tile([128, TOK], BF)
        stats_sb = per.tile([128, TOK], BF)
        o_mix = per.tile([128, 2 * TOK], BF)

        nc.gpsimd.memset(stats32[:], 0.0)
        nc.gpsimd.memset(stats_sb[:], 0.0)
        nc.vector.memset(S_bf[:], 0.0)
        nc.gpsimd.memset(v_cm[:, 0:PAD], 0.0)
        nc.gpsimd.memset(v_cm[:, PAD + TOK:PAD + TOK + PAD], 0.0)

        # ================= Stage 1+2: projections, convs, silu ================
        mid = tc.tile_pool(name="mid", bufs=1)
        midp = mid.__enter__()
        q_cm = midp.tile([128, 2 * TOK], BF)
        delta_tm = midp.tile([128, NCH * DV], BF)
        k_cm = midp.tile([128, 2 * TOK], BF)
        kb_cm = midp.tile([128, 2 * TOK], BF)
        bb = midp.tile([128, TOK], BF)

        with tc.tile_pool(name="stg1", bufs=1) as stg1:
            wproj = stg1.tile([128, 8 * 7 * 128], BF)
            nc.sync.dma_start(wproj[:], g_w[:, 0:W_WPROJ])
            cdiag = stg1.tile([128, 6 * CONV_K * 128], BF)
            for s in range(6 * CONV_K):
                build_diag(cdiag[:, s * 128:(s + 1) * 128], CP_CTAPS + s)
            qkvb = stg1.tile([128, 6 * (PAD + TOK)], BF)
            for mt in range(6):
                nc.gpsimd.memset(
                    qkvb[:, mt * (PAD + TOK): mt * (PAD + TOK) + PAD], 0.0)

            for n in range(NC4):
                xb = stg1.tile([128, 8 * 512], BF, tag="xb", bufs=2)
                for kt in range(8):
                    nc.sync.dma_start(
                        xb[:, kt * 512:(kt + 1) * 512],
                        g_xT[kt * 128:(kt + 1) * 128,
                             n * 512:(n + 1) * 512])
                pb = ps_a.tile([128, 512], FP, tag="psa")
                for kt in range(8):
                    s = kt * 7 + 6
                    nc.tensor.matmul(
                        pb[0:1, :],
                        wproj[:, s * 128: s * 128 + 1],
                        xb[:, kt * 512:(kt + 1) * 512],
                        start=(kt == 0), stop=(kt == 7),
                    )
                nc.scalar.activation(brow[0:1, n * 512:(n + 1) * 512],
                                     pb[0:1, :], AF.Sigmoid)

                for mt in range(6):
                    pt = ps_a.tile([128, 512], FP, tag="psa")
                    for kt in range(8):
                        s = kt * 7 + mt
                        nc.tensor.matmul(
                            pt[:],
                            wproj[:, s * 128:(s + 1) * 128],
                            xb[:, kt * 512:(kt + 1) * 512],
                            start=(kt == 0), stop=(kt == 7),
                        )
                    dst = qkvb[:, mt * (PAD + TOK) + PAD + n * 512:
                               mt * (PAD + TOK) + PAD + (n + 1) * 512]
                    if (mt * NC4 + n) % 2 == 0:
                        nc.scalar.copy(dst, pt[:])
                    else:
                        nc.vector.tensor_copy(dst, pt[:])

            for ci in range(NCH):
                ptt = ps_t.tile([128, 128], BF, tag="ptt")
                nc.tensor.transpose(
                    ptt[:, 0:1], brow[0:1, ci * 128:(ci + 1) * 128],
                    ident[0:1, 0:1])
                nc.scalar.copy(beta_tm[:, ci:ci + 1], ptt[:, 0:1])
            for n in range(NC4):
                nc.gpsimd.partition_broadcast(
                    bb[:, n * 512:(n + 1) * 512], brow[0:1, n * 512:(n + 1) * 512])

            for t in range(6):  # q0 q1 k0 k1 v0 v1
                for n in range(NC4):
                    pt = ps_a.tile([128, 512], FP, tag="psa")
                    base = t * (PAD + TOK) + PAD + n * 512
                    for j in range(CONV_K):
                        nc.tensor.matmul(
                            pt[:],
                            cdiag[:, (t * CONV_K + j) * 128:
                                  (t * CONV_K + j + 1) * 128],
                            qkvb[:, base - j: base - j + 512],
                            start=(j == 0), stop=(j == CONV_K - 1),
                        )
                    if t < 2:
                        dst = q_cm[:, t * TOK + n * 512: t * TOK + (n + 1) * 512]
                    elif t < 4:
                        dst = k_cm[:, (t - 2) * TOK + n * 512:
                                   (t - 2) * TOK + (n + 1) * 512]
                    else:
                        dst = v_cm[:, (t - 4) * (PAD + TOK) + PAD + n * 512:
                                   (t - 4) * (PAD + TOK) + PAD + (n + 1) * 512]
                    nc.scalar.activation(dst, pt[:], AF.Silu)

        # ================= Stage 3: l2norm of q, k; kb =================
        for t_cm in (q_cm, k_cm):
            pn = ps_a.tile([128, 512], FP, tag="psa")
            first = True
            for kt in range(2):
                sq = scr.tile([128, TOK], BF, tag="s4", bufs=2)
                nc.vector.tensor_mul(sq[:],
                                     t_cm[:, kt * TOK:(kt + 1) * TOK],
                                     t_cm[:, kt * TOK:(kt + 1) * TOK])
                for n in range(NC4):
                    nc.tensor.matmul(
                        pn[0:4, :], ones4[:, n * 4:(n + 1) * 4],
                        sq[:, n * 512:(n + 1) * 512],
                        start=first, stop=(kt == 1 and n == NC4 - 1))
                    first = False
            rn = scr.tile([4, 512], FP, tag="s1", bufs=2)
            nc.scalar.activation(rn[:], pn[0:4, :], AF.Sqrt, bias=1e-6)
            nc.vector.reciprocal(rn[:], rn[:])
            rnb = scr.tile([4, 512], BF, tag="s1", bufs=2)
            nc.vector.tensor_copy(rnb[:], rn[:])
            rnrow = scr.tile([1, TOK], BF, tag="row", bufs=1)
            nc.sync.dma_start(rnrow[0:1, :], rnb[:])
            nb = scr.tile([128, TOK], BF, tag="s4", bufs=2)
            for n in range(NC4):
                nc.gpsimd.partition_broadcast(
                    nb[:, n * 512:(n + 1) * 512], rnrow[0:1, n * 512:(n + 1) * 512])
            for kt in range(2):
                nc.vector.tensor_mul(t_cm[:, kt * TOK:(kt + 1) * TOK],
                                     t_cm[:, kt * TOK:(kt + 1) * TOK], nb[:])
        for kt in range(2):
            nc.vector.tensor_mul(kb_cm[:, kt * TOK:(kt + 1) * TOK],
                                 k_cm[:, kt * TOK:(kt + 1) * TOK], bb[:])

        # ================= Stage 4: token-major transposes =================
        mid2 = tc.tile_pool(name="mid2", bufs=1)
        midp2 = mid2.__enter__()
        k_tm = midp2.tile([128, NCH * DK], BF)
        kb_tm = midp2.tile([128, NCH * DK], BF)
        vb_tm = midp2.tile([128, NCH * DV], BF)
        for ci in range(NCH):
            bcol = beta_tm[:, ci:ci + 1]
            for kt in range(2):
                ptt = ps_t.tile([128, 128], BF, tag="ptt")
                nc.tensor.transpose(
                    ptt[:],
                    k_cm[:, kt * TOK + ci * 128: kt * TOK + (ci + 1) * 128],
                    ident[:])
                nc.scalar.copy(
                    k_tm[:, ci * DK + kt * 128: ci * DK + (kt + 1) * 128], ptt[:])
                nc.vector.tensor_scalar_mul(
                    kb_tm[:, ci * DK + kt * 128: ci * DK + (kt + 1) * 128],
                    ptt[:], bcol)
                ptv = ps_t.tile([128, 128], BF, tag="ptt")
                nc.tensor.transpose(
                    ptv[:],
                    v_cm[:, kt * (PAD + TOK) + PAD + ci * 128:
                         kt * (PAD + TOK) + PAD + (ci + 1) * 128],
                    ident[:])
                nc.scalar.activation(
                    vb_tm[:, ci * DV + kt * 128: ci * DV + (kt + 1) * 128],
                    ptv[:], AF.Copy, scale=bcol)

        # ================= Stage 5: delta-rule chunk pre =================
        u_tm = midp2.tile([128, NCH * DV], BF)
        w_cmt = midp2.tile([128, 2 * TOK], BF)
        attn_t = midp2.tile([128, NCH * 128], BF)

        for ci in range(NCH):
            pA = ps_b.tile([128, 256], FP, tag="psb")
            pAt = ps_b.tile([128, 256], FP, tag="psb")
            for kt in range(2):
                sl_k = k_cm[:, kt * TOK + ci * 128: kt * TOK + (ci + 1) * 128]
                sl_kb = kb_cm[:, kt * TOK + ci * 128: kt * TOK + (ci + 1) * 128]
                nc.tensor.matmul(pA[:, 0:128], sl_kb, sl_k,
                                 start=(kt == 0), stop=(kt == 1))
                nc.tensor.matmul(pAt[:, 0:128], sl_k, sl_kb,
                                 start=(kt == 0), stop=(kt == 1))
            Pv = scr.tile([128, 128], BF, tag="P")
            Pt = scr.tile([128, 128], BF, tag="Pt")
            nc.vector.tensor_mul(Pv[:], pA[:, 0:128], msl[:])
            nc.vector.tensor_mul(Pt[:], pAt[:, 0:128], msu[:])
            Tt = scr.tile([128, 128], BF, tag="Tt")
            nc.vector.tensor_sub(Tt[:], ident[:], Pt[:])

            pq = ps_b.tile([128, 256], FP, tag="psb")
            for kt in range(2):
                nc.tensor.matmul(
                    pq[:, 0:128],
                    k_cm[:, kt * TOK + ci * 128: kt * TOK + (ci + 1) * 128],
                    q_cm[:, kt * TOK + ci * 128: kt * TOK + (ci + 1) * 128],
                    start=(kt == 0), stop=(kt == 1))
            nc.vector.tensor_mul(attn_t[:, ci * 128:(ci + 1) * 128],
                                 pq[:, 0:128], msud[:])

            for lvl in range(6):
                psq = ps_b.tile([128, 256], FP, tag="psb")
                nc.tensor.matmul(psq[:, 0:128], Pt[:], Pv[:], start=True, stop=True)
                Pn = scr.tile([128, 128], BF, tag="P")
                nc.scalar.copy(Pn[:], psq[:, 0:128])
                if lvl < 5:
                    psq2 = ps_b.tile([128, 256], FP, tag="psb")
                    nc.tensor.matmul(psq2[:, 0:128], Pv[:], Pt[:],
                                     start=True, stop=True)
                    Ptn = scr.tile([128, 128], BF, tag="Pt")
                    nc.scalar.copy(Ptn[:], psq2[:, 0:128])
                else:
                    Ptn = Pt
                pprod = ps_b.tile([128, 256], FP, tag="psb")
                nc.tensor.matmul(pprod[:, 0:128], Pn[:], Tt[:],
                                 start=True, stop=False)
                nc.tensor.matmul(pprod[:, 0:128], ident[:], Tt[:],
                                 start=False, stop=True)
                Ttn = scr.tile([128, 128], BF, tag="Tt")
                if lvl % 2 == 0:
                    nc.vector.tensor_copy(Ttn[:], pprod[:, 0:128])
                else:
                    nc.scalar.copy(Ttn[:], pprod[:, 0:128])
                Pv, Pt, Tt = Pn, Ptn, Ttn

            pu = ps_b.tile([128, 256], FP, tag="psb")
            nc.tensor.matmul(pu[:], Tt[:], vb_tm[:, ci * DV:(ci + 1) * DV],
                             start=True, stop=True)
            nc.scalar.copy(u_tm[:, ci * DV:(ci + 1) * DV], pu[:])
            for kt in range(2):
                pw = ps_b.tile([128, 256], FP, tag="psb")
                nc.tensor.matmul(
                    pw[:, 0:128],
                    kb_tm[:, ci * DK + kt * 128: ci * DK + (kt + 1) * 128],
                    Tt[:], start=True, stop=True)
                nc.vector.tensor_copy(
                    w_cmt[:, kt * TOK + ci * 128: kt * TOK + (ci + 1) * 128],
                    pw[:, 0:128])

        # ================= Stage 6: FIR convs =================
        with tc.tile_pool(name="fir", bufs=1) as firp:
            for kt in range(2):
                fsd = firp.tile([128, FIR_S * 128], BF, tag="fsd")
                for j in range(FIR_S):
                    build_diag(fsd[:, j * 128:(j + 1) * 128],
                               CP_FSTAPS + kt * FIR_S + j)
                fld = firp.tile([128, FIR_L * 128], BF, tag="fld")
                for j in range(FIR_L):
                    build_diag(fld[:, j * 128:(j + 1) * 128],
                               CP_FLTAPS + kt * FIR_L + j)
                vbase = kt * (PAD + TOK) + PAD
                for n in range(NC4):
                    pt = ps_a.tile([128, 512], FP, tag="psa")
                    for j in range(FIR_S):
                        nc.tensor.matmul(
                            pt[:], fsd[:, j * 128:(j + 1) * 128],
                            v_cm[:, vbase + n * 512 - j: vbase + (n + 1) * 512 - j],
                            start=(j == 0), stop=(j == FIR_S - 1))
                    nc.scalar.copy(
                        fir_s[:, kt * TOK + n * 512: kt * TOK + (n + 1) * 512],
                        pt[:])
                    pt2 = ps_a.tile([128, 512], FP, tag="psa")
                    for j in range(FIR_L):
                        nc.tensor.matmul(
                            pt2[:], fld[:, j * 128:(j + 1) * 128],
                            v_cm[:, vbase + n * 512 - j: vbase + (n + 1) * 512 - j],
                            start=(j == 0), stop=(j == FIR_L - 1))
                    nc.scalar.copy(
                        fir_l[:, kt * TOK + n * 512: kt * TOK + (n + 1) * 512],
                        pt2[:])

        # ================= Stage 7: serial scan =================
        for ci in range(NCH):
            pu2 = ps_b.tile([128, 256], FP, tag="psb")
            for kt in range(2):
                nc.tensor.matmul(
                    pu2[:],
                    w_cmt[:, kt * TOK + ci * 128: kt * TOK + (ci + 1) * 128],
                    S_bf[:, kt * DV:(kt + 1) * DV],
                    start=(kt == 0), stop=(kt == 1))
            u2 = scr.tile([128, 256], BF, tag="u2")
            nc.vector.tensor_sub(u2[:], u_tm[:, ci * DV:(ci + 1) * DV], pu2[:])
            po = ps_b.tile([128, 256], FP, tag="psb")
            for kt in range(2):
                nc.tensor.matmul(
                    po[:],
                    q_cm[:, kt * TOK + ci * 128: kt * TOK + (ci + 1) * 128],
                    S_bf[:, kt * DV:(kt + 1) * DV],
                    start=(kt == 0), stop=False)
            nc.tensor.matmul(po[:], attn_t[:, ci * 128:(ci + 1) * 128], u2[:],
                             start=False, stop=True)
            nc.scalar.copy(delta_tm[:, ci * DV:(ci + 1) * DV], po[:])
            pS = ps_b.tile([128, 256], FP, tag="psb")
            nc.tensor.matmul(pS[:], k_tm[:, ci * DK: ci * DK + 128], u2[:],
                             start=True, stop=True)
            pS2 = ps_b.tile([128, 256], FP, tag="psb")
            nc.tensor.matmul(pS2[:], k_tm[:, ci * DK + 128: ci * DK + 256], u2[:],
                             start=True, stop=True)
            nc.vector.tensor_add(S_bf[:, 0:DV], S_bf[:, 0:DV], pS[:])
            nc.vector.tensor_add(S_bf[:, DV:2 * DV], S_bf[:, DV:2 * DV], pS2[:])

        for ci in range(NCH):
            for kt in range(2):
                ptt = ps_t.tile([128, 128], BF, tag="ptt")
                nc.tensor.transpose(
                    ptt[:],
                    delta_tm[:, ci * DV + kt * 128: ci * DV + (kt + 1) * 128],
                    ident[:])
                nc.scalar.copy(
                    delta_cm[:, kt * TOK + ci * 128: kt * TOK + (ci + 1) * 128],
                    ptt[:])

        # ================= Stage 8: branch stats =================
        st_f32 = scr.tile([16, 512], FP, tag="s1", bufs=2)
        branches = ((0, fir_s, 0, TOK), (1, fir_l, 0, TOK),
                    (2, delta_cm, 0, TOK), (3, v_cm, PAD, PAD + TOK))
        for n in range(NC4):
            pst = ps_a.tile([128, 512], FP, tag="psa")
            for bi, tns, off, stride in branches:
                for kt in range(2):
                    src = tns[:, kt * stride + off + n * 512:
                              kt * stride + off + (n + 1) * 512]
                    nc.tensor.matmul(
                        pst[0:48, :],
                        sumsel[:, (bi * 2) * 48:(bi * 2 + 1) * 48], src,
                        start=(bi == 0 and kt == 0), stop=False)
                    sqt = scr.tile([128, 512], BF, tag="s1", bufs=2)
                    nc.vector.tensor_mul(sqt[:], src, src)
                    nc.tensor.matmul(
                        pst[0:48, :],
                        sumsel[:, (bi * 2 + 1) * 48:(bi * 2 + 2) * 48], sqt[:],
                        start=False, stop=(bi == 3 and kt == 1))
            nc.scalar.activation(stats32[0:16, n * 512:(n + 1) * 512],
                                 pst[0:16, :], AF.Copy, scale=1.0 / DV)
            msq = scr.tile([16, 512], FP, tag="s1", bufs=2)
            nc.vector.tensor_mul(msq[:], stats32[0:16, n * 512:(n + 1) * 512],
                                 stats32[0:16, n * 512:(n + 1) * 512])
            nc.vector.scalar_tensor_tensor(
                st_f32[:], pst[32:48, :], 1.0 / DV, msq[:],
                op0=OP.mult, op1=OP.subtract)
            nc.vector.tensor_scalar_max(st_f32[:], st_f32[:], 1e-6)
            nc.scalar.activation(stats32[32:48, n * 512:(n + 1) * 512],
                                 st_f32[:], AF.Sqrt)

        mid2.__exit__(None, None, None)
        mid.__exit__(None, None, None)

        # ============ Stage 9: stats AllReduce (all tokens) ========
        st_in = dram.tile([32, TOK], BF)
        st_out = dram.tile([32, TOK], BF)
        nc.sync.dma_start(st_in[0:16, :], stats32[0:16, :])
        nc.sync.dma_start(st_in[16:32, :], stats32[32:48, :])
        nc.gpsimd.collective_compute(
            "AllReduce", mybir.AluOpType.add, replica_groups=rg4,
            ins=[st_in[:]], outs=[st_out[:]])
        nc.sync.dma_start(stats_sb[0:32, :], st_out[:])

        # ====== Stage 10: gate MLP (4 of 16 hidden tiles, all tokens) ========
        with tc.tile_pool(name="tail", bufs=1) as tail, \
             tc.tile_pool(name="w1p", bufs=3) as w1p:
            w1t4 = tail.tile([128, 4 * 1152], BF)
            for mt2 in range(4):
                nc.sync.dma_start(w1t4[:, mt2 * 1152:(mt2 + 1) * 1152],
                                  g_w1[mt2 * 128:(mt2 + 1) * 128, :])
            h1 = tail.tile([128, 4 * TOK], BF)
            for n in range(NC4):
                xb2 = w1p.tile([128, 8 * 512], BF, tag="xb2", bufs=2)
                for kt in range(8):
                    nc.sync.dma_start(
                        xb2[:, kt * 512:(kt + 1) * 512],
                        g_xT[kt * 128:(kt + 1) * 128, n * 512:(n + 1) * 512])
                for mt2 in range(4):
                    pt = ps_a.tile([128, 512], FP, tag="psa")
                    for kt in range(9):
                        rhs = (xb2[:, kt * 512:(kt + 1) * 512] if kt < 8
                               else stats_sb[:, n * 512:(n + 1) * 512])
                        nc.tensor.matmul(
                            pt[:], w1t4[:, mt2 * 1152 + kt * 128:
                                        mt2 * 1152 + (kt + 1) * 128],
                            rhs, start=(kt == 0), stop=(kt == 8))
                    nc.scalar.activation(
                        h1[:, mt2 * TOK + n * 512: mt2 * TOK + (n + 1) * 512],
                        pt[:], AF.Gelu, bias=b1c[:, mt2:mt2 + 1])

            pls = tail.tile([16, 4 * 512], FP)
            for n in range(NC4):
                pl = ps_a.tile([128, 512], FP, tag="psa")
                for mt2 in range(4):
                    nc.tensor.matmul(
                        pl[0:16, :], w2my[:, mt2 * 16:(mt2 + 1) * 16],
                        h1[:, mt2 * TOK + n * 512: mt2 * TOK + (n + 1) * 512],
                        start=(mt2 == 0), stop=(mt2 == 3))
                nc.scalar.copy(pls[:, n * 512:(n + 1) * 512], pl[0:16, :])

            pl_in = dram.tile([64, 512], FP)
            pl_out = dram.tile([16, 512], FP)
            for n in range(NC4):
                nc.sync.dma_start(pl_in[n * 16:(n + 1) * 16, :],
                                  pls[:, n * 512:(n + 1) * 512])
            nc.gpsimd.collective_compute(
                "ReduceScatter", mybir.AluOpType.add, replica_groups=rg4,
                ins=[pl_in[:]], outs=[pl_out[:]])
            plq = tail.tile([16, 512], FP)
            nc.sync.dma_start(plq[:], pl_out[:])

            expt = tail.tile([16, 512], BF)
            nc.scalar.activation(expt[:], plq[:], AF.Exp, bias=b2c[:, 0:1])
            pg = ps_b.tile([128, 256], FP, tag="psb")
            nc.tensor.matmul(pg[0:16, :], bsum[:], expt[:, 0:256],
                             start=True, stop=True)
            pg2 = ps_b.tile([128, 256], FP, tag="psb")
            nc.tensor.matmul(pg2[0:16, :], bsum[:], expt[:, 256:512],
                             start=True, stop=True)
            gsum = tail.tile([16, 512], FP)
            nc.scalar.copy(gsum[:, 0:256], pg[0:16, :])
            nc.scalar.copy(gsum[:, 256:512], pg2[0:16, :])
            nc.vector.reciprocal(gsum[:], gsum[:])
            wg = tail.tile([16, 512], BF)
            nc.vector.tensor_mul(wg[:], expt[:], gsum[:])

            # ============ Stage 11: gate AllGather + extraction ============
            wg_in = dram.tile([16, 512], BF)
            wg_out = dram.tile([64, 512], BF)
            nc.sync.dma_start(wg_in[:], wg[:])
            nc.gpsimd.collective_compute(
                "AllGather", mybir.AluOpType.bypass, replica_groups=rg4,
                ins=[wg_in[:]], outs=[wg_out[:]])
            wrow = tail.tile([1, 4 * TOK], BF)
            for g in range(4):
                gt = w1p.tile([16, 512], BF, tag="gath")
                nc.sync.dma_start(gt[:], wg_out[g * 16:(g + 1) * 16, :])
                pw4 = ps_b.tile([128, 256], FP, tag="psb")
                nc.tensor.matmul(pw4[:], hsel[:], gt[:, 0:256],
                                 start=True, stop=True)
                pw42 = ps_b.tile([128, 256], FP, tag="psb")
                nc.tensor.matmul(pw42[:], hsel[:], gt[:, 256:512],
                                 start=True, stop=True)
                for r in range(4):
                    nc.scalar.copy(
                        wrow[0:1, r * TOK + g * 512: r * TOK + g * 512 + 256],
                        pw4[32 * r:32 * r + 1, :])
                    nc.scalar.copy(
                        wrow[0:1, r * TOK + g * 512 + 256: r * TOK + (g + 1) * 512],
                        pw42[32 * r:32 * r + 1, :])

            wb4 = tail.tile([128, 4 * TOK], BF)
            for j in range(4):
                for n in range(NC4):
                    nc.gpsimd.partition_broadcast(
                        wb4[:, j * TOK + n * 512: j * TOK + (n + 1) * 512],
                        wrow[0:1, j * TOK + n * 512: j * TOK + (n + 1) * 512])

            # ============ Stage 12: mix + RMSNorm ============
            for kt in range(2):
                t1 = o_mix[:, kt * TOK:(kt + 1) * TOK]
                t2 = scr.tile([128, TOK], BF, tag="s4", bufs=2)
                nc.vector.tensor_mul(t1, wb4[:, 0:TOK],
                                     fir_s[:, kt * TOK:(kt + 1) * TOK])
                nc.vector.tensor_mul(t2[:], wb4[:, TOK:2 * TOK],
                                     fir_l[:, kt * TOK:(kt + 1) * TOK])
                nc.vector.tensor_add(t1, t1, t2[:])
                nc.vector.tensor_mul(t2[:], wb4[:, 2 * TOK:3 * TOK],
                                     delta_cm[:, kt * TOK:(kt + 1) * TOK])
                nc.vector.tensor_add(t1, t1, t2[:])
                nc.vector.tensor_mul(
                    t2[:], wb4[:, 3 * TOK:4 * TOK],
                    v_cm[:, kt * (PAD + TOK) + PAD: kt * (PAD + TOK) + PAD + TOK])
                nc.vector.tensor_add(t1, t1, t2[:])

            prms = ps_a.tile([128, 512], FP, tag="psa")
            first = True
            for n in range(NC4):
                for kt in range(2):
                    sqm = scr.tile([128, 512], BF, tag="s1", bufs=2)
                    src = o_mix[:, kt * TOK + n * 512: kt * TOK + (n + 1) * 512]
                    nc.vector.tensor_mul(sqm[:], src, src)
                    nc.tensor.matmul(prms[0:4, :], ones4[:, n * 4:(n + 1) * 4],
                                     sqm[:], start=first,
                                     stop=(n == NC4 - 1 and kt == 1))
                    first = False
            rms = scr.tile([4, 512], FP, tag="s1", bufs=2)
            nc.scalar.activation(rms[:], prms[0:4, :], AF.Sqrt,
                                 scale=1.0 / DV, bias=1e-5)
            nc.vector.reciprocal(rms[:], rms[:])
            rmsb = scr.tile([4, 512], BF, tag="s1", bufs=2)
            nc.vector.tensor_copy(rmsb[:], rms[:])
            rmsrow = scr.tile([1, TOK], BF, tag="row", bufs=1)
            nc.sync.dma_start(rmsrow[0:1, :], rmsb[:])
            rb = scr.tile([128, TOK], BF, tag="s4", bufs=2)
            for n in range(NC4):
                nc.gpsimd.partition_broadcast(
                    rb[:, n * 512:(n + 1) * 512], rmsrow[0:1, n * 512:(n + 1) * 512])
            for kt in range(2):
                nc.vector.scalar_tensor_tensor(
                    o_mix[:, kt * TOK:(kt + 1) * TOK],
                    o_mix[:, kt * TOK:(kt + 1) * TOK],
                    onwc[:, kt:kt + 1], rb[:], op0=OP.mult, op1=OP.mult)

            # ===== Stage 13+14: partial output projection + ReduceScatter ====
            rs_in = dram.tile([2048, 1024], BF)
            rs_out = dram.tile([512, 1024], BF)
            for mt in range(16):
                ost = w1p.tile([128, 1024], BF, tag="ost")
                for n2 in range(2):
                    pt = ps_a.tile([128, 512], FP, tag="psa")
                    for kt in range(2):
                        nc.tensor.matmul(
                            pt[:],
                            o_mix[:, kt * TOK + mt * 128: kt * TOK + (mt + 1) * 128],
                            wos[:, kt * 1024 + n2 * 512: kt * 1024 + (n2 + 1) * 512],
                            start=(kt == 0), stop=(kt == 1))
                    nc.scalar.copy(ost[:, n2 * 512:(n2 + 1) * 512], pt[:])
                nc.sync.dma_start(rs_in[mt * 128:(mt + 1) * 128, :], ost[:])
            nc.gpsimd.collective_compute(
                "ReduceScatter", mybir.AluOpType.add, replica_groups=rg4,
                ins=[rs_in[:]], outs=[rs_out[:]])
            nc.sync.dma_start(d_out.ap(), rs_out[:])

    nc.compile()
    return nc


def _host_prep(inputs):
    """Build the global (concat-over-cores along axis 0) input arrays."""
    x = np.asarray(inputs["hidden_states"], F32NP)
    Wq = np.asarray(inputs["Wq"], F32NP)
    Wk = np.asarray(inputs["Wk"], F32NP)
    Wv = np.asarray(inputs["Wv"], F32NP)
    Wb = np.asarray(inputs["Wb"], F32NP)
    cqw = np.asarray(inputs["conv_q_w"], F32NP)
    ckw = np.asarray(inputs["conv_k_w"], F32NP)
    cvw = np.asarray(inputs["conv_v_w"], F32NP)
    fsw = np.asarray(inputs["fir_short_w"], F32NP).reshape(NH * DV, FIR_S)
    flw = np.asarray(inputs["fir_long_w"], F32NP).reshape(NH * DV, FIR_L)
    w1 = np.asarray(inputs["mlp_w1"], F32NP)
    b1 = np.asarray(inputs["mlp_b1"], F32NP)
    w2 = np.asarray(inputs["mlp_w2"], F32NP)
    b2 = np.asarray(inputs["mlp_b2"], F32NP)
    glt = np.asarray(inputs["gate_log_temp"], np.float64)
    onw = np.asarray(inputs["o_norm_w"], F32NP)
    Wo = np.asarray(inputs["Wo"], F32NP)

    temp = (np.log1p(np.exp(glt)) + 1e-4).astype(F32NP)
    tcol = np.repeat(temp, 4)
    w2f = (w2 / tcol[None, :]).astype(F32NP)
    b2f = (b2 / tcol).astype(F32NP)

    # x channel-quarter slices, already in per-core concat order
    xq = np.ascontiguousarray(np.transpose(x, (0, 2, 1))) \
        .reshape(8 * 256, TOK).astype(BF16NP)

    # w1 with stats rows permuted to device layout + pad to 1152
    w1p = np.zeros((1152, MLP_H), F32NP)
    w1p[0:1024] = w1[0:1024]
    bi = np.arange(4)[:, None]
    hh = np.arange(4)[None, :]
    w1p[(1024 + bi * 4 + hh).ravel()] = w1[(1024 + bi * 8 + hh).ravel()]
    w1p[(1040 + bi * 4 + hh).ravel()] = w1[(1028 + bi * 8 + hh).ravel()]
    # w1s block (mt, kt) = w1p block (kt, mt)
    w1s = w1p.reshape(9, 128, 16, 128).transpose(2, 1, 0, 3) \
        .reshape(16 * 128, 9 * 128).astype(BF16NP)
    # core (b, g) ships w1s rows [g*512 + b*256 : +256]
    w1q = np.ascontiguousarray(
        w1s.reshape(4, 2, 256, 1152).transpose(1, 0, 2, 3).reshape(2048, 1152))

    idx = np.arange(128)
    ident = np.zeros((128, 128), F32NP)
    ident[idx, idx] = 1
    ii = idx[:, None]
    jj = idx[None, :]
    msl = (ii > jj).astype(F32NP)
    msu = (ii < jj).astype(F32NP)
    msud = (ii <= jj).astype(F32NP)
    ones4 = np.zeros((128, 16), F32NP)
    for n in range(4):
        ones4[:, n * 4 + n] = 1
    bsum = np.zeros((16, 16), F32NP)
    for kk in range(16):
        for mm in range(16):
            if kk // 4 == mm // 4:
                bsum[kk, mm] = 1

    whalf = np.zeros((2, 4, 64, W_WH), BF16NP)
    for h in range(4):
        hsl = slice(h * 256, (h + 1) * 256)
        wh_h = np.zeros((128, W_WH), F32NP)
        wproj_full = np.zeros((1024, 896), F32NP)
        wproj_full[:, 0:256] = Wq[:, hsl]
        wproj_full[:, 256:512] = Wk[:, hsl]
        wproj_full[:, 512:768] = Wv[:, hsl]
        wproj_full[:, 768] = Wb[:, h]
        wh_h[:, 0:W_WPROJ] = wproj_full.reshape(8, 128, 7, 128) \
            .transpose(1, 0, 2, 3).reshape(128, W_WPROJ)
        wh_h[:, W_WPROJ:W_WPROJ + W_WOS] = Wo[hsl].reshape(2, 128, 1024) \
            .transpose(1, 0, 2).reshape(128, 2048)

        cp = wh_h[:, W_WPROJ + W_WOS:]
        cp[:, CP_IDENT:CP_IDENT + 128] = ident
        cp[:, CP_MSL:CP_MSL + 128] = msl
        cp[:, CP_MSU:CP_MSU + 128] = msu
        cp[:, CP_MSUD:CP_MSUD + 128] = msud
        cp[:, CP_ONES4:CP_ONES4 + 16] = ones4
        for bi2 in range(4):
            cp[:, CP_SUMSEL + (bi2 * 2) * 48 + bi2 * 4 + h] = 1
            cp[:, CP_SUMSEL + (bi2 * 2 + 1) * 48 + 32 + bi2 * 4 + h] = 1
        for mt2 in range(4):
            cp[:, CP_W2 + mt2 * 16: CP_W2 + (mt2 + 1) * 16] = \
                w2f[(4 * h + mt2) * 128:(4 * h + mt2 + 1) * 128, :]
        cp[0:16, CP_BSUM:CP_BSUM + 16] = bsum
        for r in range(4):
            cp[4 * h + r, CP_HSEL + 32 * r] = 1
        for ti, cw in enumerate((cqw[hsl], ckw[hsl], cvw[hsl])):
            for t2 in range(2):
                for j in range(CONV_K):
                    cp[:, CP_CTAPS + (ti * 2 + t2) * CONV_K + (CONV_K - 1 - j)] \
                        = cw[t2 * 128:(t2 + 1) * 128, j]
        for kt in range(2):
            for j in range(FIR_S):
                cp[:, CP_FSTAPS + kt * FIR_S + (FIR_S - 1 - j)] = \
                    fsw[hsl][kt * 128:(kt + 1) * 128, j]
            for j in range(FIR_L):
                cp[:, CP_FLTAPS + kt * FIR_L + (FIR_L - 1 - j)] = \
                    flw[hsl][kt * 128:(kt + 1) * 128, j]
        whb = wh_h.astype(BF16NP)
        whalf[0, h] = whb[0:64]
        whalf[1, h] = whb[64:128]
    whalf = whalf.reshape(8 * 64, W_WH)

    fp = np.zeros((4, 128, 7), F32NP)
    for g in range(4):
        for j in range(4):
            fp[g, :, j] = b1[(4 * g + j) * 128:(4 * g + j + 1) * 128]
    fp[:, 0:16, 4] = b2f
    fp[:, :, 5] = onw[0:128]
    fp[:, :, 6] = onw[128:256]
    fpack = np.concatenate([fp, fp], 0).reshape(8, 128 * 7).astype(BF16NP)

    blob = np.empty((8, NBLOB), BF16NP)
    blob[:, O_XQ:O_WH] = xq.reshape(8, -1)
    blob[:, O_WH:O_W1] = whalf.reshape(8, -1)
    blob[:, O_W1:O_FP] = w1q.reshape(8, -1)
    blob[:, O_FP:] = fpack
    return {"blob": blob}


def _get_nc():
    if "nc" not in _CACHE:
        _CACHE["nc"] = _build_program()
    return _CACHE["nc"]


def _make_sharding():
    import jax
    from jax.sharding import Mesh, NamedSharding, PartitionSpec

    devices = jax.devices()[:8]
    mesh = Mesh(np.asarray(devices), ("core",))
    return mesh, NamedSharding(mesh, PartitionSpec("core"))


def _make_compiled(nc):
    """AOT-compile the sharded bass_exec callable (and an on-device zeros
    producer for the donated output buffer)."""
    import jax
    import jax.numpy as jnp
    import concourse.mybir as mybir
    from jax.sharding import PartitionSpec
    from jax.experimental.shard_map import shard_map
    from concourse import bass2jax
    from concourse.bass2jax import _bass_exec_p, partition_id_tensor

    bass2jax.install_neuronx_cc_hook()
    partition_name = (nc.partition_id_tensor.name
                      if nc.partition_id_tensor else None)
    in_names, out_names, out_avals = [], [], []
    for alloc in nc.m.functions[0].allocations:
        if not isinstance(alloc, mybir.MemoryLocationSet):
            continue
        name = alloc.memorylocations[0].name
        if alloc.kind == "ExternalInput":
            if name != partition_name:
                in_names.append(name)
        elif alloc.kind == "ExternalOutput":
            out_names.append(name)
            out_avals.append(jax.core.ShapedArray(
                tuple(alloc.tensor_shape), mybir.dt.np(alloc.dtype)))
    n_params = len(in_names)
    all_in_names = list(in_names) + out_names
    if partition_name is not None:
        all_in_names.append(partition_name)

    def _body(*args):
        operands = list(args)
        if partition_name is not None:
            operands.append(partition_id_tensor())
        return tuple(_bass_exec_p.bind(
            *operands, out_avals=tuple(out_avals), in_names=tuple(all_in_names),
            out_names=tuple(out_names), lowering_input_output_aliases=(),
            sim_require_finite=True, sim_require_nnan=True, nc=nc))

    mesh, sh = _make_sharding()
    donate = tuple(range(n_params, n_params + len(out_names)))
    sharded = jax.jit(
        shard_map(_body, mesh=mesh,
                  in_specs=(PartitionSpec("core"),) * (n_params + len(out_names)),
                  out_specs=(PartitionSpec("core"),) * len(out_names),
                  check_rep=False),
        donate_argnums=donate, keep_unused=True)

    def g_spec(name):
        for alloc in nc.m.functions[0].allocations:
            if (isinstance(alloc, mybir.MemoryLocationSet)
                    and alloc.memorylocations[0].name == name):
                shp = tuple(alloc.tensor_shape)
                return jax.ShapeDtypeStruct(
                    (8 * shp[0],) + shp[1:], mybir.dt.np(alloc.dtype),
                    sharding=sh)
        raise KeyError(name)

    specs = [g_spec(n) for n in in_names] + [g_spec(n) for n in out_names]
    compiled = sharded.lower(*specs).compile()
    zeros = jax.jit(
        lambda: jnp.zeros((8 * TSL, H), jnp.bfloat16),
        out_shardings=sh).lower().compile()
    return {"compiled": compiled, "zeros": zeros,
            "in_names": in_names, "out_names": out_names}


_INIT = {}


def _bg_init():
    try:
        nc = _get_nc()
        _INIT.update(_make_compiled(nc))
    except BaseException as e:  # noqa: BLE001 - reraised in kernel()
        _INIT["err"] = e


_BG = threading.Thread(target=_bg_init, daemon=True)
_BG.start()


def kernel(**inputs):
    globs = _host_prep(inputs)

    import jax

    _mesh, sh = _make_sharding()
    put = {k: jax.device_put(v, sh) for k, v in globs.items()}
    _BG.join()
    if "err" in _INIT:
        raise _INIT["err"]

    args = [put[n] for n in _INIT["in_names"]] + [_INIT["zeros"]()]
    out_arrs = _INIT["compiled"](*args)
    o = np.asarray(out_arrs[0]).astype(F32NP).reshape(8, TSL, H)
    full = np.empty((B, L, H), F32NP)
    for c in range(8):
        full[c // 4, (c % 4) * TSL:(c % 4 + 1) * TSL] = o[c]
    return full


def run_traced(inputs, trace=True):
    """Dev helper: run via run_bass_kernel_spmd to capture a profile."""
    from concourse.bass_utils import run_bass_kernel_spmd

    nc = _get_nc()
    globs = _host_prep(inputs)
    in_maps = [
        {"blob": np.ascontiguousarray(globs["blob"][c:c + 1])}
        for c in range(8)
    ]
    res = run_bass_kernel_spmd(nc, in_maps, core_ids=list(range(8)), trace=trace)
    out = np.zeros((B, L, H), F32NP)
    for c in range(8):
        out[c // 4, (c % 4) * TSL:(c % 4 + 1) * TSL] = \
            np.asarray(res.results[c]["out"], F32NP)
    return out, res


# revision 19
# speedup vs baseline: 1.2134x; 1.1613x over previous
"""DeltaNet block as a Bass/Tile SPMD kernel on 8 TRN2 NeuronCores.

Sharding: one (batch, head) pair per core (B=2 x NH=4 = 8 cores).

Host->device traffic is minimized: each core uploads only a 1/4-channel
slice of its batch's x^T, half of its head's packed weights (the batch-pair
core uploads the other half), and a quarter of its group's gate-MLP w1 slice.
On-device AllGathers (4-way for x, pair-wise for weights) reconstruct the
full operands over NeuronLink. Depthwise-conv/FIR diagonal matrices are
built on device from compact tap vectors (ident * tap column).

Per core: q/k/v/beta projections (head slice) + causal convs + silu, l2norm,
chunkwise delta rule (C=128 chunks, product-form unit-triangular inverse),
FIR convs (PE diagonal-matmul), branch stats, a 4-core AllReduce of the
stats rows, the gate MLP (4 of 16 hidden tiles x all tokens, with a
partial-logit ReduceScatter), AllGather of gate weights, channel-major
4-way mix + RMSNorm, the output projection and a ReduceScatter that both
sums heads and splits tokens. Output is returned bf16 and upcast on host.

All matmul operands are bf16 (fp32 PSUM accumulation).
"""

import sys
import threading

import numpy as np

if "/opt/trn_rl_repo" not in sys.path:
    sys.path.insert(0, "/opt/trn_rl_repo")

import ml_dtypes

BF16NP = ml_dtypes.bfloat16
F32NP = np.float32

B, L, H = 2, 2048, 1024
NH, DK, DV = 4, 256, 256
CONV_K, FIR_S, FIR_L = 4, 3, 63
MLP_H = 2 * H
C = 128
NCH = L // C
PAD = 64
TOK = L
TSL = L // 4
NC4 = TOK // 512

# packed-weight column offsets inside cpack
CP_IDENT = 0
CP_MSL = 128
CP_MSU = 256
CP_MSUD = 384
CP_ONES4 = 512
CP_SUMSEL = 528          # 8 blocks x 48
CP_W2 = 912              # 4 blocks x 16
CP_BSUM = 976            # [16,16]
CP_HSEL = 992            # [16,128]
CP_CTAPS = 1120          # 6 tensors x 4 taps
CP_FSTAPS = 1144         # 2 kt x 3 taps
CP_FLTAPS = 1150         # 2 kt x 63 taps
W_CPACK = 1280
W_WPROJ = 7168
W_WOS = 2048
W_WH = W_WPROJ + W_WOS + W_CPACK  # 10496

# single fused per-core input blob (bf16 elements)
O_XQ = 0
O_WH = O_XQ + 256 * TOK          # 524288
O_W1 = O_WH + 64 * W_WH          # 1196032
O_FP = O_W1 + 256 * 1152         # 1490944
NBLOB = O_FP + 128 * 7           # 1491840

_CACHE = {}


def _build_program():
    from contextlib import ExitStack

    import concourse.bacc as bacc
    import concourse.mybir as mybir
    import concourse.tile as tile

    dt = mybir.dt
    BF = dt.bfloat16
    FP = dt.float32
    AF = mybir.ActivationFunctionType
    OP = mybir.AluOpType

    nc = bacc.Bacc("TRN2", target_bir_lowering=False, debug=False, num_devices=8)

    for v in (1e-6, 1e-5):
        t = nc.alloc_sbuf_tensor(f"const-float32-{v}", [128, 1], FP)
        nc.gpsimd.memset(t.ap(), v)
        nc.const_aps.aps[(FP, v)] = t.ap()
    nc.all_engine_barrier()

    d_blob = nc.dram_tensor("blob", [1, NBLOB], BF, kind="ExternalInput")
    d_out = nc.dram_tensor("out", [TSL, H], BF, kind="ExternalOutput")

    rg4 = [[0, 1, 2, 3], [4, 5, 6, 7]]
    rgp = [[0, 4], [1, 5], [2, 6], [3, 7]]

    with tile.TileContext(nc) as tc, ExitStack() as es:
        cst = es.enter_context(tc.tile_pool(name="cst", bufs=1))
        per = es.enter_context(tc.tile_pool(name="per", bufs=1))
        scr = es.enter_context(tc.tile_pool(name="scr", bufs=3))
        ps_a = es.enter_context(tc.tile_pool(name="ps_a", bufs=2, space="PSUM"))
        ps_b = es.enter_context(tc.tile_pool(name="ps_b", bufs=4, space="PSUM"))
        ps_t = es.enter_context(tc.tile_pool(name="ps_t", bufs=2, space="PSUM"))
        dram = es.enter_context(tc.tile_pool(name="dram", bufs=1, space="DRAM"))

        # ============ Stage 0: assemble inputs via on-device AllGather =======
        i_xq = dram.tile([256, TOK], BF)
        i_wh = dram.tile([64, W_WH], BF)
        i_w1 = dram.tile([256, 1152], BF)
        g_xT = dram.tile([1024, TOK], BF)
        g_w = dram.tile([128, W_WH], BF)
        g_w1 = dram.tile([512, 1152], BF)
        nc.sync.dma_start(i_xq[:], d_blob.ap()[0:1, O_XQ:O_WH])
        nc.sync.dma_start(i_wh[:], d_blob.ap()[0:1, O_WH:O_W1])
        nc.sync.dma_start(i_w1[:], d_blob.ap()[0:1, O_W1:O_FP])
        nc.gpsimd.collective_compute(
            "AllGather", mybir.AluOpType.bypass, replica_groups=rg4,
            ins=[i_xq[:]], outs=[g_xT[:]])
        nc.gpsimd.collective_compute(
            "AllGather", mybir.AluOpType.bypass, replica_groups=rgp,
            ins=[i_wh[:]], outs=[g_w[:]])
        nc.gpsimd.collective_compute(
            "AllGather", mybir.AluOpType.bypass, replica_groups=rgp,
            ins=[i_w1[:]], outs=[g_w1[:]])

        cpack = cst.tile([128, W_CPACK], BF)
        nc.sync.dma_start(cpack[:], g_w[:, W_WPROJ + W_WOS: W_WH])
        fpbf = cst.tile([128, 7], BF)
        nc.sync.dma_start(fpbf[:], d_blob.ap()[0:1, O_FP:NBLOB])
        fpack = cst.tile([128, 7], FP)
        nc.vector.tensor_copy(fpack[:], fpbf[:])
        wos = cst.tile([128, W_WOS], BF)
        nc.sync.dma_start(wos[:], g_w[:, W_WPROJ: W_WPROJ + W_WOS])

        ident = cpack[:, CP_IDENT:CP_IDENT + 128]
        msl = cpack[:, CP_MSL:CP_MSL + 128]
        msu = cpack[:, CP_MSU:CP_MSU + 128]
        msud = cpack[:, CP_MSUD:CP_MSUD + 128]
        ones4 = cpack[:, CP_ONES4:CP_ONES4 + 16]
        sumsel = cpack[:, CP_SUMSEL:CP_SUMSEL + 384]
        w2my = cpack[:, CP_W2:CP_W2 + 64]
        bsum = cpack[0:16, CP_BSUM:CP_BSUM + 16]
        hsel = cpack[0:16, CP_HSEL:CP_HSEL + 128]
        b1c = fpack[:, 0:4]
        b2c = fpack[0:16, 4:5]
        onwc = fpack[:, 5:7]

        # fp32 copy of the tap columns (scalar operands must be fp32)
        tapf = cst.tile([128, W_CPACK - CP_CTAPS], FP)
        nc.vector.tensor_copy(tapf[:], cpack[:, CP_CTAPS:W_CPACK])

        def build_diag(dst, col):
            # dst[128,128] = diag(tap column `col` of cpack); alternate engines
            scol = tapf[:, col - CP_CTAPS: col - CP_CTAPS + 1]
            if col % 2 == 0:
                nc.scalar.activation(dst, ident, AF.Copy, scale=scol)
            else:
                nc.vector.tensor_scalar_mul(dst, ident, scol)

        v_cm = per.tile([128, 2 * (PAD + TOK)], BF)
        fir_s = per.tile([128, 2 * TOK], BF)
        fir_l = per.tile([128, 2 * TOK], BF)
        delta_cm = per.tile([128, 2 * TOK], BF)
        brow = per.tile([1, TOK], BF)
        beta_tm = per.tile([128, NCH], FP)
        S_bf = per.tile([128, 2 * DV], BF)
        stats32 = per.tile([128, TOK], BF)
        stats_sb = per.tile([128, TOK], BF)
        o_mix = per.tile([128, 2 * TOK], BF)

        nc.gpsimd.memset(stats32[:], 0.0)
        nc.gpsimd.memset(stats_sb[:], 0.0)
        nc.vector.memset(S_bf[:], 0.0)
        nc.gpsimd.memset(v_cm[:, 0:PAD], 0.0)
        nc.gpsimd.memset(v_cm[:, PAD + TOK:PAD + TOK + PAD], 0.0)

        # ================= Stage 1+2: projections, convs, silu ================
        mid = tc.tile_pool(name="mid", bufs=1)
        midp = mid.__enter__()
        q_cm = midp.tile([128, 2 * TOK], BF)
        delta_tm = midp.tile([128, NCH * DV], BF)
        k_cm = midp.tile([128, 2 * TOK], BF)
        kb_cm = midp.tile([128, 2 * TOK], BF)
        bb = midp.tile([128, TOK], BF)

        with tc.tile_pool(name="stg1", bufs=1) as stg1:
            wproj = stg1.tile([128, 8 * 7 * 128], BF)
            nc.sync.dma_start(wproj[:], g_w[:, 0:W_WPROJ])
            cdiag = stg1.tile([128, 6 * CONV_K * 128], BF)
            for s in range(6 * CONV_K):
                build_diag(cdiag[:, s * 128:(s + 1) * 128], CP_CTAPS + s)
            qkvb = stg1.tile([128, 6 * (PAD + TOK)], BF)
            for mt in range(6):
                nc.gpsimd.memset(
                    qkvb[:, mt * (PAD + TOK): mt * (PAD + TOK) + PAD], 0.0)

            for n in range(NC4):
                xb = stg1.tile([128, 8 * 512], BF, tag="xb", bufs=2)
                for kt in range(8):
                    nc.sync.dma_start(
                        xb[:, kt * 512:(kt + 1) * 512],
                        g_xT[kt * 128:(kt + 1) * 128,
                             n * 512:(n + 1) * 512])
                pb = ps_a.tile([128, 512], FP, tag="psa")
                for kt in range(8):
                    s = kt * 7 + 6
                    nc.tensor.matmul(
                        pb[0:1, :],
                        wproj[:, s * 128: s * 128 + 1],
                        xb[:, kt * 512:(kt + 1) * 512],
                        start=(kt == 0), stop=(kt == 7),
                    )
                nc.scalar.activation(brow[0:1, n * 512:(n + 1) * 512],
                                     pb[0:1, :], AF.Sigmoid)

                for mt in range(6):
                    pt = ps_a.tile([128, 512], FP, tag="psa")
                    for kt in range(8):
                        s = kt * 7 + mt
                        nc.tensor.matmul(
                            pt[:],
                            wproj[:, s * 128:(s + 1) * 128],
                            xb[:, kt * 512:(kt + 1) * 512],
                            start=(kt == 0), stop=(kt == 7),
                        )
                    dst = qkvb[:, mt * (PAD + TOK) + PAD + n * 512:
                               mt * (PAD + TOK) + PAD + (n + 1) * 512]
                    if (mt * NC4 + n) % 2 == 0:
                        nc.scalar.copy(dst, pt[:])
                    else:
                        nc.vector.tensor_copy(dst, pt[:])

            for ci in range(NCH):
                ptt = ps_t.tile([128, 128], BF, tag="ptt")
                nc.tensor.transpose(
                    ptt[:, 0:1], brow[0:1, ci * 128:(ci + 1) * 128],
                    ident[0:1, 0:1])
                nc.scalar.copy(beta_tm[:, ci:ci + 1], ptt[:, 0:1])
            for n in range(NC4):
                nc.gpsimd.partition_broadcast(
                    bb[:, n * 512:(n + 1) * 512], brow[0:1, n * 512:(n + 1) * 512])

            for t in range(6):  # q0 q1 k0 k1 v0 v1
                for n in range(NC4):
                    pt = ps_a.tile([128, 512], FP, tag="psa")
                    base = t * (PAD + TOK) + PAD + n * 512
                    for j in range(CONV_K):
                        nc.tensor.matmul(
                            pt[:],
                            cdiag[:, (t * CONV_K + j) * 128:
                                  (t * CONV_K + j + 1) * 128],
                            qkvb[:, base - j: base - j + 512],
                            start=(j == 0), stop=(j == CONV_K - 1),
                        )
                    if t < 2:
                        dst = q_cm[:, t * TOK + n * 512: t * TOK + (n + 1) * 512]
                    elif t < 4:
                        dst = k_cm[:, (t - 2) * TOK + n * 512:
                                   (t - 2) * TOK + (n + 1) * 512]
                    else:
                        dst = v_cm[:, (t - 4) * (PAD + TOK) + PAD + n * 512:
                                   (t - 4) * (PAD + TOK) + PAD + (n + 1) * 512]
                    nc.scalar.activation(dst, pt[:], AF.Silu)

        # ================= Stage 3: l2norm of q, k; kb =================
        for t_cm in (q_cm, k_cm):
            pn = ps_a.tile([128, 512], FP, tag="psa")
            first = True
            for kt in range(2):
                sq = scr.tile([128, TOK], BF, tag="s4", bufs=2)
                nc.vector.tensor_mul(sq[:],
                                     t_cm[:, kt * TOK:(kt + 1) * TOK],
                                     t_cm[:, kt * TOK:(kt + 1) * TOK])
                for n in range(NC4):
                    nc.tensor.matmul(
                        pn[0:4, :], ones4[:, n * 4:(n + 1) * 4],
                        sq[:, n * 512:(n + 1) * 512],
                        start=first, stop=(kt == 1 and n == NC4 - 1))
                    first = False
            rn = scr.tile([4, 512], FP, tag="s1", bufs=2)
            nc.scalar.activation(rn[:], pn[0:4, :], AF.Sqrt, bias=1e-6)
            nc.vector.reciprocal(rn[:], rn[:])
            rnb = scr.tile([4, 512], BF, tag="s1", bufs=2)
            nc.vector.tensor_copy(rnb[:], rn[:])
            rnrow = scr.tile([1, TOK], BF, tag="row", bufs=1)
            nc.sync.dma_start(rnrow[0:1, :], rnb[:])
            nb = scr.tile([128, TOK], BF, tag="s4", bufs=2)
            for n in range(NC4):
                nc.gpsimd.partition_broadcast(
                    nb[:, n * 512:(n + 1) * 512], rnrow[0:1, n * 512:(n + 1) * 512])
            for kt in range(2):
                nc.vector.tensor_mul(t_cm[:, kt * TOK:(kt + 1) * TOK],
                                     t_cm[:, kt * TOK:(kt + 1) * TOK], nb[:])
        for kt in range(2):
            nc.vector.tensor_mul(kb_cm[:, kt * TOK:(kt + 1) * TOK],
                                 k_cm[:, kt * TOK:(kt + 1) * TOK], bb[:])

        # ================= Stage 4: token-major transposes =================
        mid2 = tc.tile_pool(name="mid2", bufs=1)
        midp2 = mid2.__enter__()
        k_tm = midp2.tile([128, NCH * DK], BF)
        kb_tm = midp2.tile([128, NCH * DK], BF)
        vb_tm = midp2.tile([128, NCH * DV], BF)
        for ci in range(NCH):
            bcol = beta_tm[:, ci:ci + 1]
            for kt in range(2):
                ptt = ps_t.tile([128, 128], BF, tag="ptt")
                nc.tensor.transpose(
                    ptt[:],
                    k_cm[:, kt * TOK + ci * 128: kt * TOK + (ci + 1) * 128],
                    ident[:])
                nc.scalar.copy(
                    k_tm[:, ci * DK + kt * 128: ci * DK + (kt + 1) * 128], ptt[:])
                nc.vector.tensor_scalar_mul(
                    kb_tm[:, ci * DK + kt * 128: ci * DK + (kt + 1) * 128],
                    ptt[:], bcol)
                ptv = ps_t.tile([128, 128], BF, tag="ptt")
                nc.tensor.transpose(
                    ptv[:],
                    v_cm[:, kt * (PAD + TOK) + PAD + ci * 128:
                         kt * (PAD + TOK) + PAD + (ci + 1) * 128],
                    ident[:])
                nc.scalar.activation(
                    vb_tm[:, ci * DV + kt * 128: ci * DV + (kt + 1) * 128],
                    ptv[:], AF.Copy, scale=bcol)

        # ================= Stage 5: delta-rule chunk pre =================
        u_tm = midp2.tile([128, NCH * DV], BF)
        w_cmt = midp2.tile([128, 2 * TOK], BF)
        attn_t = midp2.tile([128, NCH * 128], BF)

        for ci in range(NCH):
            pA = ps_b.tile([128, 256], FP, tag="psb")
            pAt = ps_b.tile([128, 256], FP, tag="psb")
            for kt in range(2):
                sl_k = k_cm[:, kt * TOK + ci * 128: kt * TOK + (ci + 1) * 128]
                sl_kb = kb_cm[:, kt * TOK + ci * 128: kt * TOK + (ci + 1) * 128]
                nc.tensor.matmul(pA[:, 0:128], sl_kb, sl_k,
                                 start=(kt == 0), stop=(kt == 1))
                nc.tensor.matmul(pAt[:, 0:128], sl_k, sl_kb,
                                 start=(kt == 0), stop=(kt == 1))
            Pv = scr.tile([128, 128], BF, tag="P")
            Pt = scr.tile([128, 128], BF, tag="Pt")
            nc.vector.tensor_mul(Pv[:], pA[:, 0:128], msl[:])
            nc.vector.tensor_mul(Pt[:], pAt[:, 0:128], msu[:])
            Tt = scr.tile([128, 128], BF, tag="Tt")
            nc.vector.tensor_sub(Tt[:], ident[:], Pt[:])

            pq = ps_b.tile([128, 256], FP, tag="psb")
            for kt in range(2):
                nc.tensor.matmul(
                    pq[:, 0:128],
                    k_cm[:, kt * TOK + ci * 128: kt * TOK + (ci + 1) * 128],
                    q_cm[:, kt * TOK + ci * 128: kt * TOK + (ci + 1) * 128],
                    start=(kt == 0), stop=(kt == 1))
            nc.vector.tensor_mul(attn_t[:, ci * 128:(ci + 1) * 128],
                                 pq[:, 0:128], msud[:])

            for lvl in range(6):
                psq = ps_b.tile([128, 256], FP, tag="psb")
                nc.tensor.matmul(psq[:, 0:128], Pt[:], Pv[:], start=True, stop=True)
                Pn = scr.tile([128, 128], BF, tag="P")
                nc.scalar.copy(Pn[:], psq[:, 0:128])
                if lvl < 5:
                    psq2 = ps_b.tile([128, 256], FP, tag="psb")
                    nc.tensor.matmul(psq2[:, 0:128], Pv[:], Pt[:],
                                     start=True, stop=True)
                    Ptn = scr.tile([128, 128], BF, tag="Pt")
                    nc.scalar.copy(Ptn[:], psq2[:, 0:128])
                else:
                    Ptn = Pt
                pprod = ps_b.tile([128, 256], FP, tag="psb")
                nc.tensor.matmul(pprod[:, 0:128], Pn[:], Tt[:],
                                 start=True, stop=False)
                nc.tensor.matmul(pprod[:, 0:128], ident[:], Tt[:],
                                 start=False, stop=True)
                Ttn = scr.tile([128, 128], BF, tag="Tt")
                if lvl % 2 == 0:
                    nc.vector.tensor_copy(Ttn[:], pprod[:, 0:128])
                else:
                    nc.scalar.copy(Ttn[:], pprod[:, 0:128])
                Pv, Pt, Tt = Pn, Ptn, Ttn

            pu = ps_b.tile([128, 256], FP, tag="psb")
            nc.tensor.matmul(pu[:], Tt[:], vb_tm[:, ci * DV:(ci + 1) * DV],
                             start=True, stop=True)
            nc.scalar.copy(u_tm[:, ci * DV:(ci + 1) * DV], pu[:])
            for kt in range(2):
                pw = ps_b.tile([128, 256], FP, tag="psb")
                nc.tensor.matmul(
                    pw[:, 0:128],
                    kb_tm[:, ci * DK + kt * 128: ci * DK + (kt + 1) * 128],
                    Tt[:], start=True, stop=True)
                nc.vector.tensor_copy(
                    w_cmt[:, kt * TOK + ci * 128: kt * TOK + (ci + 1) * 128],
                    pw[:, 0:128])

        # ================= Stage 6: FIR convs =================
        with tc.tile_pool(name="fir", bufs=1) as firp:
            for kt in range(2):
                fsd = firp.tile([128, FIR_S * 128], BF, tag="fsd")
                for j in range(FIR_S):
                    build_diag(fsd[:, j * 128:(j + 1) * 128],
                               CP_FSTAPS + kt * FIR_S + j)
                fld = firp.tile([128, FIR_L * 128], BF, tag="fld")
                for j in range(FIR_L):
                    build_diag(fld[:, j * 128:(j + 1) * 128],
                               CP_FLTAPS + kt * FIR_L + j)
                vbase = kt * (PAD + TOK) + PAD
                for n in range(NC4):
                    pt = ps_a.tile([128, 512], FP, tag="psa")
                    for j in range(FIR_S):
                        nc.tensor.matmul(
                            pt[:], fsd[:, j * 128:(j + 1) * 128],
                            v_cm[:, vbase + n * 512 - j: vbase + (n + 1) * 512 - j],
                            start=(j == 0), stop=(j == FIR_S - 1))
                    nc.scalar.copy(
                        fir_s[:, kt * TOK + n * 512: kt * TOK + (n + 1) * 512],
                        pt[:])
                    pt2 = ps_a.tile([128, 512], FP, tag="psa")
                    for j in range(FIR_L):
                        nc.tensor.matmul(
                            pt2[:], fld[:, j * 128:(j + 1) * 128],
                            v_cm[:, vbase + n * 512 - j: vbase + (n + 1) * 512 - j],
                            start=(j == 0), stop=(j == FIR_L - 1))
                    nc.scalar.copy(
                        fir_l[:, kt * TOK + n * 512: kt * TOK + (n + 1) * 512],
                        pt2[:])

        # ================= Stage 7: serial scan =================
        for ci in range(NCH):
            pu2 = ps_b.tile([128, 256], FP, tag="psb")
            for kt in range(2):
                nc.tensor.matmul(
                    pu2[:],
                    w_cmt[:, kt * TOK + ci * 128: kt * TOK + (ci + 1) * 128],
                    S_bf[:, kt * DV:(kt + 1) * DV],
                    start=(kt == 0), stop=(kt == 1))
            u2 = scr.tile([128, 256], BF, tag="u2")
            nc.vector.tensor_sub(u2[:], u_tm[:, ci * DV:(ci + 1) * DV], pu2[:])
            po = ps_b.tile([128, 256], FP, tag="psb")
            for kt in range(2):
                nc.tensor.matmul(
                    po[:],
                    q_cm[:, kt * TOK + ci * 128: kt * TOK + (ci + 1) * 128],
                    S_bf[:, kt * DV:(kt + 1) * DV],
                    start=(kt == 0), stop=False)
            nc.tensor.matmul(po[:], attn_t[:, ci * 128:(ci + 1) * 128], u2[:],
                             start=False, stop=True)
            nc.scalar.copy(delta_tm[:, ci * DV:(ci + 1) * DV], po[:])
            pS = ps_b.tile([128, 256], FP, tag="psb")
            nc.tensor.matmul(pS[:], k_tm[:, ci * DK: ci * DK + 128], u2[:],
                             start=True, stop=True)
            pS2 = ps_b.tile([128, 256], FP, tag="psb")
            nc.tensor.matmul(pS2[:], k_tm[:, ci * DK + 128: ci * DK + 256], u2[:],
                             start=True, stop=True)
            nc.vector.tensor_add(S_bf[:, 0:DV], S_bf[:, 0:DV], pS[:])
            nc.vector.tensor_add(S_bf[:, DV:2 * DV], S_bf[:, DV:2 * DV], pS2[:])

        for ci in range(NCH):
            for kt in range(2):
                ptt = ps_t.tile([128, 128], BF, tag="ptt")
                nc.tensor.transpose(
                    ptt[:],
                    delta_tm[:, ci * DV + kt * 128: ci * DV + (kt + 1) * 128],
                    ident[:])
                nc.scalar.copy(
                    delta_cm[:, kt * TOK + ci * 128: kt * TOK + (ci + 1) * 128],
                    ptt[:])

        # ================= Stage 8: branch stats =================
        st_f32 = scr.tile([16, 512], FP, tag="s1", bufs=2)
        branches = ((0, fir_s, 0, TOK), (1, fir_l, 0, TOK),
                    (2, delta_cm, 0, TOK), (3, v_cm, PAD, PAD + TOK))
        for n in range(NC4):
            pst = ps_a.tile([128, 512], FP, tag="psa")
            for bi, tns, off, stride in branches:
                for kt in range(2):
                    src = tns[:, kt * stride + off + n * 512:
                              kt * stride + off + (n + 1) * 512]
                    nc.tensor.matmul(
                        pst[0:48, :],
                        sumsel[:, (bi * 2) * 48:(bi * 2 + 1) * 48], src,
                        start=(bi == 0 and kt == 0), stop=False)
                    sqt = scr.tile([128, 512], BF, tag="s1", bufs=2)
                    nc.vector.tensor_mul(sqt[:], src, src)
                    nc.tensor.matmul(
                        pst[0:48, :],
                        sumsel[:, (bi * 2 + 1) * 48:(bi * 2 + 2) * 48], sqt[:],
                        start=False, stop=(bi == 3 and kt == 1))
            nc.scalar.activation(stats32[0:16, n * 512:(n + 1) * 512],
                                 pst[0:16, :], AF.Copy, scale=1.0 / DV)
            msq = scr.tile([16, 512], FP, tag="s1", bufs=2)
            nc.vector.tensor_mul(msq[:], stats32[0:16, n * 512:(n + 1) * 512],
                                 stats32[0:16, n * 512:(n + 1) * 512])
            nc.vector.scalar_tensor_tensor(
                st_f32[:], pst[32:48, :], 1.0 / DV, msq[:],
                op0=OP.mult, op1=OP.subtract)
            nc.vector.tensor_scalar_max(st_f32[:], st_f32[:], 1e-6)
            nc.scalar.activation(stats32[32:48, n * 512:(n + 1) * 512],
                                 st_f32[:], AF.Sqrt)

        mid2.__exit__(None, None, None)
        mid.__exit__(None, None, None)

        # ============ Stage 9: stats AllReduce (all tokens) ========
        st_in = dram.tile([32, TOK], BF)
        st_out = dram.tile([32, TOK], BF)
        nc.sync.dma_start(st_in[0:16, :], stats32[0:16, :])
        nc.sync.dma_start(st_in[16:32, :], stats32[32:48, :])
        nc.gpsimd.collective_compute(
            "AllReduce", mybir.AluOpType.add, replica_groups=rg4,
            ins=[st_in[:]], outs=[st_out[:]])
        nc.sync.dma_start(stats_sb[0:32, :], st_out[:])

        # ====== Stage 10: gate MLP (4 of 16 hidden tiles, all tokens) ========
        with tc.tile_pool(name="tail", bufs=1) as tail, \
             tc.tile_pool(name="w1p", bufs=3) as w1p:
            w1t4 = tail.tile([128, 4 * 1152], BF)
            for mt2 in range(4):
                nc.sync.dma_start(w1t4[:, mt2 * 1152:(mt2 + 1) * 1152],
                                  g_w1[mt2 * 128:(mt2 + 1) * 128, :])
            h1 = tail.tile([128, 4 * TOK], BF)
            for n in range(NC4):
                xb2 = w1p.tile([128, 8 * 512], BF, tag="xb2", bufs=2)
                for kt in range(8):
                    nc.sync.dma_start(
                        xb2[:, kt * 512:(kt + 1) * 512],
                        g_xT[kt * 128:(kt + 1) * 128, n * 512:(n + 1) * 512])
                for mt2 in range(4):
                    pt = ps_a.tile([128, 512], FP, tag="psa")
                    for kt in range(9):
                        rhs = (xb2[:, kt * 512:(kt + 1) * 512] if kt < 8
                               else stats_sb[:, n * 512:(n + 1) * 512])
                        nc.tensor.matmul(
                            pt[:], w1t4[:, mt2 * 1152 + kt * 128:
                                        mt2 * 1152 + (kt + 1) * 128],
                            rhs, start=(kt == 0), stop=(kt == 8))
                    nc.scalar.activation(
                        h1[:, mt2 * TOK + n * 512: mt2 * TOK + (n + 1) * 512],
                        pt[:], AF.Gelu, bias=b1c[:, mt2:mt2 + 1])

            pls = tail.tile([16, 4 * 512], FP)
            for n in range(NC4):
                pl = ps_a.tile([128, 512], FP, tag="psa")
                for mt2 in range(4):
                    nc.tensor.matmul(
                        pl[0:16, :], w2my[:, mt2 * 16:(mt2 + 1) * 16],
                        h1[:, mt2 * TOK + n * 512: mt2 * TOK + (n + 1) * 512],
                        start=(mt2 == 0), stop=(mt2 == 3))
                nc.scalar.copy(pls[:, n * 512:(n + 1) * 512], pl[0:16, :])

            pl_in = dram.tile([64, 512], FP)
            pl_out = dram.tile([16, 512], FP)
            for n in range(NC4):
                nc.sync.dma_start(pl_in[n * 16:(n + 1) * 16, :],
                                  pls[:, n * 512:(n + 1) * 512])
            nc.gpsimd.collective_compute(
                "ReduceScatter", mybir.AluOpType.add, replica_groups=rg4,
                ins=[pl_in[:]], outs=[pl_out[:]])
            plq = tail.tile([16, 512], FP)
            nc.sync.dma_start(plq[:], pl_out[:])

            expt = tail.tile([16, 512], BF)
            nc.scalar.activation(expt[:], plq[:], AF.Exp, bias=b2c[:, 0:1])
            pg = ps_b.tile([128, 256], FP, tag="psb")
            nc.tensor.matmul(pg[0:16, :], bsum[:], expt[:, 0:256],
                             start=True, stop=True)
            pg2 = ps_b.tile([128, 256], FP, tag="psb")
            nc.tensor.matmul(pg2[0:16, :], bsum[:], expt[:, 256:512],
                             start=True, stop=True)
            gsum = tail.tile([16, 512], FP)
            nc.scalar.copy(gsum[:, 0:256], pg[0:16, :])
            nc.scalar.copy(gsum[:, 256:512], pg2[0:16, :])
            nc.vector.reciprocal(gsum[:], gsum[:])
            wg = tail.tile([16, 512], BF)
            nc.vector.tensor_mul(wg[:], expt[:], gsum[:])

            # ============ Stage 11: gate AllGather + extraction ============
            wg_in = dram.tile([16, 512], BF)
            wg_out = dram.tile([64, 512], BF)
            nc.sync.dma_start(wg_in[:], wg[:])
            nc.gpsimd.collective_compute(
                "AllGather", mybir.AluOpType.bypass, replica_groups=rg4,
                ins=[wg_in[:]], outs=[wg_out[:]])
            wrow = tail.tile([1, 4 * TOK], BF)
            for g in range(4):
                gt = w1p.tile([16, 512], BF, tag="gath")
                nc.sync.dma_start(gt[:], wg_out[g * 16:(g + 1) * 16, :])
                pw4 = ps_b.tile([128, 256], FP, tag="psb")
                nc.tensor.matmul(pw4[:], hsel[:], gt[:, 0:256],
                                 start=True, stop=True)
                pw42 = ps_b.tile([128, 256], FP, tag="psb")
                nc.tensor.matmul(pw42[:], hsel[:], gt[:, 256:512],
                                 start=True, stop=True)
                for r in range(4):
                    nc.scalar.copy(
                        wrow[0:1, r * TOK + g * 512: r * TOK + g * 512 + 256],
                        pw4[32 * r:32 * r + 1, :])
                    nc.scalar.copy(
                        wrow[0:1, r * TOK + g * 512 + 256: r * TOK + (g + 1) * 512],
                        pw42[32 * r:32 * r + 1, :])

            wb4 = tail.tile([128, 4 * TOK], BF)
            for j in range(4):
                for n in range(NC4):
                    nc.gpsimd.partition_broadcast(
                        wb4[:, j * TOK + n * 512: j * TOK + (n + 1) * 512],
                        wrow[0:1, j * TOK + n * 512: j * TOK + (n + 1) * 512])

            # ============ Stage 12: mix + RMSNorm ============
            for kt in range(2):
                t1 = o_mix[:, kt * TOK:(kt + 1) * TOK]
                t2 = scr.tile([128, TOK], BF, tag="s4", bufs=2)
                nc.vector.tensor_mul(t1, wb4[:, 0:TOK],
                                     fir_s[:, kt * TOK:(kt + 1) * TOK])
                nc.vector.tensor_mul(t2[:], wb4[:, TOK:2 * TOK],
                                     fir_l[:, kt * TOK:(kt + 1) * TOK])
                nc.vector.tensor_add(t1, t1, t2[:])
                nc.vector.tensor_mul(t2[:], wb4[:, 2 * TOK:3 * TOK],
                                     delta_cm[:, kt * TOK:(kt + 1) * TOK])
                nc.vector.tensor_add(t1, t1, t2[:])
                nc.vector.tensor_mul(
                    t2[:], wb4[:, 3 * TOK:4 * TOK],
                    v_cm[:, kt * (PAD + TOK) + PAD: kt * (PAD + TOK) + PAD + TOK])
                nc.vector.tensor_add(t1, t1, t2[:])

            prms = ps_a.tile([128, 512], FP, tag="psa")
            first = True
            for n in range(NC4):
                for kt in range(2):
                    sqm = scr.tile([128, 512], BF, tag="s1", bufs=2)
                    src = o_mix[:, kt * TOK + n * 512: kt * TOK + (n + 1) * 512]
                    nc.vector.tensor_mul(sqm[:], src, src)
                    nc.tensor.matmul(prms[0:4, :], ones4[:, n * 4:(n + 1) * 4],
                                     sqm[:], start=first,
                                     stop=(n == NC4 - 1 and kt == 1))
                    first = False
            rms = scr.tile([4, 512], FP, tag="s1", bufs=2)
            nc.scalar.activation(rms[:], prms[0:4, :], AF.Sqrt,
                                 scale=1.0 / DV, bias=1e-5)
            nc.vector.reciprocal(rms[:], rms[:])
            rmsb = scr.tile([4, 512], BF, tag="s1", bufs=2)
            nc.vector.tensor_copy(rmsb[:], rms[:])
            rmsrow = scr.tile([1, TOK], BF, tag="row", bufs=1)
            nc.sync.dma_start(rmsrow[0:1, :], rmsb[:])
            rb = scr.tile([128, TOK], BF, tag="s4", bufs=2)
            for n in range(NC4):
                nc.gpsimd.partition_broadcast(
                    rb[:, n * 512:(n + 1) * 512], rmsrow[0:1, n * 512:(n + 1) * 512])
            for kt in range(2):
                nc.vector.scalar_tensor_tensor(
                    o_mix[:, kt * TOK:(kt + 1) * TOK],
                    o_mix[:, kt * TOK:(kt + 1) * TOK],
                    onwc[:, kt:kt + 1], rb[:], op0=OP.mult, op1=OP.mult)

            # ===== Stage 13+14: partial output projection + ReduceScatter ====
            rs_in = dram.tile([2048, 1024], BF)
            rs_out = dram.tile([512, 1024], BF)
            for mt in range(16):
                ost = w1p.tile([128, 1024], BF, tag="ost")
                for n2 in range(2):
                    pt = ps_a.tile([128, 512], FP, tag="psa")
                    for kt in range(2):
                        nc.tensor.matmul(
                            pt[:],
                            o_mix[:, kt * TOK + mt * 128: kt * TOK + (mt + 1) * 128],
                            wos[:, kt * 1024 + n2 * 512: kt * 1024 + (n2 + 1) * 512],
                            start=(kt == 0), stop=(kt == 1))
                    nc.scalar.copy(ost[:, n2 * 512:(n2 + 1) * 512], pt[:])
                nc.sync.dma_start(rs_in[mt * 128:(mt + 1) * 128, :], ost[:])
            nc.gpsimd.collective_compute(
                "ReduceScatter", mybir.AluOpType.add, replica_groups=rg4,
                ins=[rs_in[:]], outs=[rs_out[:]])
            nc.sync.dma_start(d_out.ap(), rs_out[:])

    nc.compile()
    return nc


def _host_prep(inputs):
    """Build the global (concat-over-cores along axis 0) input arrays."""
    x = np.asarray(inputs["hidden_states"], F32NP)
    Wq = np.asarray(inputs["Wq"], F32NP)
    Wk = np.asarray(inputs["Wk"], F32NP)
    Wv = np.asarray(inputs["Wv"], F32NP)
    Wb = np.asarray(inputs["Wb"], F32NP)
    cqw = np.asarray(inputs["conv_q_w"], F32NP)
    ckw = np.asarray(inputs["conv_k_w"], F32NP)
    cvw = np.asarray(inputs["conv_v_w"], F32NP)
    fsw = np.asarray(inputs["fir_short_w"], F32NP).reshape(NH * DV, FIR_S)
    flw = np.asarray(inputs["fir_long_w"], F32NP).reshape(NH * DV, FIR_L)
    w1 = np.asarray(inputs["mlp_w1"], F32NP)
    b1 = np.asarray(inputs["mlp_b1"], F32NP)
    w2 = np.asarray(inputs["mlp_w2"], F32NP)
    b2 = np.asarray(inputs["mlp_b2"], F32NP)
    glt = np.asarray(inputs["gate_log_temp"], np.float64)
    onw = np.asarray(inputs["o_norm_w"], F32NP)
    Wo = np.asarray(inputs["Wo"], F32NP)

    temp = (np.log1p(np.exp(glt)) + 1e-4).astype(F32NP)
    tcol = np.repeat(temp, 4)
    w2f = (w2 / tcol[None, :]).astype(F32NP)
    b2f = (b2 / tcol).astype(F32NP)

    # x channel-quarter slices, already in per-core concat order
    xq = np.ascontiguousarray(np.transpose(x, (0, 2, 1))) \
        .reshape(8 * 256, TOK).astype(BF16NP)

    # w1 with stats rows permuted to device layout + pad to 1152
    w1p = np.zeros((1152, MLP_H), F32NP)
    w1p[0:1024] = w1[0:1024]
    bi = np.arange(4)[:, None]
    hh = np.arange(4)[None, :]
    w1p[(1024 + bi * 4 + hh).ravel()] = w1[(1024 + bi * 8 + hh).ravel()]
    w1p[(1040 + bi * 4 + hh).ravel()] = w1[(1028 + bi * 8 + hh).ravel()]
    # w1s block (mt, kt) = w1p block (kt, mt)
    w1s = w1p.reshape(9, 128, 16, 128).transpose(2, 1, 0, 3) \
        .reshape(16 * 128, 9 * 128).astype(BF16NP)
    # core (b, g) ships w1s rows [g*512 + b*256 : +256]
    w1q = np.ascontiguousarray(
        w1s.reshape(4, 2, 256, 1152).transpose(1, 0, 2, 3).reshape(2048, 1152))

    idx = np.arange(128)
    ident = np.zeros((128, 128), F32NP)
    ident[idx, idx] = 1
    ii = idx[:, None]
    jj = idx[None, :]
    msl = (ii > jj).astype(F32NP)
    msu = (ii < jj).astype(F32NP)
    msud = (ii <= jj).astype(F32NP)
    ones4 = np.zeros((128, 16), F32NP)
    for n in range(4):
        ones4[:, n * 4 + n] = 1
    bsum = np.zeros((16, 16), F32NP)
    for kk in range(16):
        for mm in range(16):
            if kk // 4 == mm // 4:
                bsum[kk, mm] = 1

    whalf = np.zeros((2, 4, 64, W_WH), BF16NP)
    for h in range(4):
        hsl = slice(h * 256, (h + 1) * 256)
        wh_h = np.zeros((128, W_WH), F32NP)
        wproj_full = np.zeros((1024, 896), F32NP)
        wproj_full[:, 0:256] = Wq[:, hsl]
        wproj_full[:, 256:512] = Wk[:, hsl]
        wproj_full[:, 512:768] = Wv[:, hsl]
        wproj_full[:, 768] = Wb[:, h]
        wh_h[:, 0:W_WPROJ] = wproj_full.reshape(8, 128, 7, 128) \
            .transpose(1, 0, 2, 3).reshape(128, W_WPROJ)
        wh_h[:, W_WPROJ:W_WPROJ + W_WOS] = Wo[hsl].reshape(2, 128, 1024) \
            .transpose(1, 0, 2).reshape(128, 2048)

        cp = wh_h[:, W_WPROJ + W_WOS:]
        cp[:, CP_IDENT:CP_IDENT + 128] = ident
        cp[:, CP_MSL:CP_MSL + 128] = msl
        cp[:, CP_MSU:CP_MSU + 128] = msu
        cp[:, CP_MSUD:CP_MSUD + 128] = msud
        cp[:, CP_ONES4:CP_ONES4 + 16] = ones4
        for bi2 in range(4):
            cp[:, CP_SUMSEL + (bi2 * 2) * 48 + bi2 * 4 + h] = 1
            cp[:, CP_SUMSEL + (bi2 * 2 + 1) * 48 + 32 + bi2 * 4 + h] = 1
        for mt2 in range(4):
            cp[:, CP_W2 + mt2 * 16: CP_W2 + (mt2 + 1) * 16] = \
                w2f[(4 * h + mt2) * 128:(4 * h + mt2 + 1) * 128, :]
        cp[0:16, CP_BSUM:CP_BSUM + 16] = bsum
        for r in range(4):
            cp[4 * h + r, CP_HSEL + 32 * r] = 1
        for ti, cw in enumerate((cqw[hsl], ckw[hsl], cvw[hsl])):
            for t2 in range(2):
                for j in range(CONV_K):
                    cp[:, CP_CTAPS + (ti * 2 + t2) * CONV_K + (CONV_K - 1 - j)] \
                        = cw[t2 * 128:(t2 + 1) * 128, j]
        for kt in range(2):
            for j in range(FIR_S):
                cp[:, CP_FSTAPS + kt * FIR_S + (FIR_S - 1 - j)] = \
                    fsw[hsl][kt * 128:(kt + 1) * 128, j]
            for j in range(FIR_L):
                cp[:, CP_FLTAPS + kt * FIR_L + (FIR_L - 1 - j)] = \
                    flw[hsl][kt * 128:(kt + 1) * 128, j]
        whb = wh_h.astype(BF16NP)
        whalf[0, h] = whb[0:64]
        whalf[1, h] = whb[64:128]
    whalf = whalf.reshape(8 * 64, W_WH)

    fp = np.zeros((4, 128, 7), F32NP)
    for g in range(4):
        for j in range(4):
            fp[g, :, j] = b1[(4 * g + j) * 128:(4 * g + j + 1) * 128]
    fp[:, 0:16, 4] = b2f
    fp[:, :, 5] = onw[0:128]
    fp[:, :, 6] = onw[128:256]
    fpack = np.concatenate([fp, fp], 0).reshape(8, 128 * 7).astype(BF16NP)

    blob = np.empty((8, NBLOB), BF16NP)
    blob[:, O_XQ:O_WH] = xq.reshape(8, -1)
    blob[:, O_WH:O_W1] = whalf.reshape(8, -1)
    blob[:, O_W1:O_FP] = w1q.reshape(8, -1)
    blob[:, O_FP:] = fpack
    return {"blob": blob}


def _get_nc():
    if "nc" not in _CACHE:
        _CACHE["nc"] = _build_program()
    return _CACHE["nc"]


def _make_sharding():
    import jax
    from jax.sharding import Mesh, NamedSharding, PartitionSpec

    devices = jax.devices()[:8]
    mesh = Mesh(np.asarray(devices), ("core",))
    return mesh, NamedSharding(mesh, PartitionSpec("core"))


def _make_compiled(nc):
    """AOT-compile the sharded bass_exec callable (and an on-device zeros
    producer for the donated output buffer)."""
    import jax
    import jax.numpy as jnp
    import concourse.mybir as mybir
    from jax.sharding import PartitionSpec
    from jax.experimental.shard_map import shard_map
    from concourse import bass2jax
    from concourse.bass2jax import _bass_exec_p, partition_id_tensor

    bass2jax.install_neuronx_cc_hook()
    partition_name = (nc.partition_id_tensor.name
                      if nc.partition_id_tensor else None)
    in_names, out_names, out_avals = [], [], []
    for alloc in nc.m.functions[0].allocations:
        if not isinstance(alloc, mybir.MemoryLocationSet):
            continue
        name = alloc.memorylocations[0].name
        if alloc.kind == "ExternalInput":
            if name != partition_name:
                in_names.append(name)
        elif alloc.kind == "ExternalOutput":
            out_names.append(name)
            out_avals.append(jax.core.ShapedArray(
                tuple(alloc.tensor_shape), mybir.dt.np(alloc.dtype)))
    n_params = len(in_names)
    all_in_names = list(in_names) + out_names
    if partition_name is not None:
        all_in_names.append(partition_name)

    def _body(*args):
        operands = list(args)
        if partition_name is not None:
            operands.append(partition_id_tensor())
        return tuple(_bass_exec_p.bind(
            *operands, out_avals=tuple(out_avals), in_names=tuple(all_in_names),
            out_names=tuple(out_names), lowering_input_output_aliases=(),
            sim_require_finite=True, sim_require_nnan=True, nc=nc))

    mesh, sh = _make_sharding()
    donate = tuple(range(n_params, n_params + len(out_names)))
    sharded = jax.jit(
        shard_map(_body, mesh=mesh,
                  in_specs=(PartitionSpec("core"),) * (n_params + len(out_names)),
                  out_specs=(PartitionSpec("core"),) * len(out_names),
                  check_rep=False),
        donate_argnums=donate, keep_unused=True)

    def g_spec(name):
        for alloc in nc.m.functions[0].allocations:
            if (isinstance(alloc, mybir.MemoryLocationSet)
                    and alloc.memorylocations[0].name == name):
                shp = tuple(alloc.tensor_shape)
                return jax.ShapeDtypeStruct(
                    (8 * shp[0],) + shp[1:], mybir.dt.np(alloc.dtype),
                    sharding=sh)
        raise KeyError(name)

    specs = [g_spec(n) for n in in_names] + [g_spec(n) for n in out_names]
    compiled = sharded.lower(*specs).compile()
    zeros = jax.jit(
        lambda: jnp.zeros((8 * TSL, H), jnp.bfloat16),
        out_shardings=sh).lower().compile()
    return {"compiled": compiled, "zeros": zeros,
            "in_names": in_names, "out_names": out_names}


_INIT = {}


def _bg_init():
    try:
        nc = _get_nc()
        _INIT.update(_make_compiled(nc))
        # launch the zero-output producer now: warms device/NEFF execution
        # paths in the background and yields the donated buffer for the call
        _INIT["zeros_arr"] = _INIT["zeros"]()
    except BaseException as e:  # noqa: BLE001 - reraised in kernel()
        _INIT["err"] = e


_BG = threading.Thread(target=_bg_init, daemon=True)
_BG.start()


def kernel(**inputs):
    globs = _host_prep(inputs)

    import jax

    _mesh, sh = _make_sharding()
    put = {k: jax.device_put(v, sh) for k, v in globs.items()}
    _BG.join()
    if "err" in _INIT:
        raise _INIT["err"]

    zeros_arr = _INIT.pop("zeros_arr", None)
    if zeros_arr is None:  # repeat call: the previous buffer was donated
        zeros_arr = _INIT["zeros"]()
    args = [put[n] for n in _INIT["in_names"]] + [zeros_arr]
    out_arrs = _INIT["compiled"](*args)
    o = np.asarray(out_arrs[0]).astype(F32NP).reshape(8, TSL, H)
    full = np.empty((B, L, H), F32NP)
    for c in range(8):
        full[c // 4, (c % 4) * TSL:(c % 4 + 1) * TSL] = o[c]
    return full


def run_traced(inputs, trace=True):
    """Dev helper: run via run_bass_kernel_spmd to capture a profile."""
    from concourse.bass_utils import run_bass_kernel_spmd

    nc = _get_nc()
    globs = _host_prep(inputs)
    in_maps = [
        {"blob": np.ascontiguousarray(globs["blob"][c:c + 1])}
        for c in range(8)
    ]
    res = run_bass_kernel_spmd(nc, in_maps, core_ids=list(range(8)), trace=trace)
    out = np.zeros((B, L, H), F32NP)
    for c in range(8):
        out[c // 4, (c % 4) * TSL:(c % 4 + 1) * TSL] = \
            np.asarray(res.results[c]["out"], F32NP)
    return out, res
